# revision 27
# baseline (speedup 1.0000x reference)
"""MeshPotential (P3M-style) Trainium2 kernel — banded-spectrum version.

Key physics: with atomic smearing 0.4 the k-space kernel G ~ exp(-0.0079 n^2)
is < 1e-7 outside integer frequencies |n| <= 32.  So only a 64 x 64 x 32
band of the 256^3 rfft spectrum matters (verified: truncation rel err 4e-6).

Per-core (8 cores SPMD, core = (channel, ky-half)) the pipeline is fully
analytic in y and z (per-atom structure factors, host-precomputed) and a
dense 256-point DFT in x only:

  P12  spread     : R(kzri, ky | x) = sum_slots a*wx*[SzR|SzI] (x) Sy
  T1   transpose  : [kzri, x] -> [x, kzri] blocks (PE transpose)
  P3   x-DFT + G  : X(kx, ky, kzri) = F_x R ;  X *= G   (banded kx: 64)
  P4   x-inverse  : V(x, ky, kzri)  = F_x^H X
  T2   transpose  : [x, kzri] -> [kzri, x] blocks
  P56  gather     : U(slot, ky) = M^T V ; pot(slot|x) = sum_ky SyPack * U

All matmul operands are bf16 (PSUM accumulates fp32).  Whole spectral cube
lives in SBUF (~5 MB); no DRAM round trips.  Host folds bin slots back to
atoms and sums the two ky-half cores per channel.
"""

import os

import numpy as np
import ml_dtypes

import concourse.bass as bass
import concourse.mybir as mybir
import concourse.tile as tile
from concourse import bacc
from concourse.bass_utils import run_bass_kernel_spmd

F32 = mybir.dt.float32
BF16 = mybir.dt.bfloat16
BFNP = ml_dtypes.bfloat16

NS = 256
BK = 64            # kx / ky band size (freqs 0..31, -32..-1)
KZB = 32           # kz band size (0..31)
KRIB = 2 * KZB     # [Re | Im] packed kz
KYH = 32           # ky values per core (half of band)
N_CORES = 8
SMEARING = 0.4
BOX_REF = None     # general cell handled via inv_cell in host_prep

_cache = {}


def build_program(C):
    XP = 128 // C                  # x cells per spread/gather sub-group
    G2 = NS // (2 * XP)            # gather batches (2 groups each)
    nc = bacc.Bacc(None, target_bir_lowering=False, debug=False)
    dp = lambda name, shape, dt=BF16: nc.declare_dram_parameter(
        name, list(shape), dt, isOutput=False)
    ST = max(1, 128 // (2 * C))    # x cells stacked per spread matmul
    XG = 16                        # x cells per spread DMA group
    NPAIR = XG // ST
    SW = 64 + 32 * ST
    spx = dp("spx", (NS // XG, 2 * C * ST, NPAIR, SW))   # [L-stack | blockdiag R]
    gx = dp("gx", (G2, BK, 2, 2, XP * C))          # [M1-all-xi | M2-all-xi]
    gy = dp("gy", (G2, 128, 2, 2, KYH, XP))        # diag-masked SyPack
    fxc = dp("fxc", (NS, BK))
    fxs = dp("fxs", (NS, BK))
    fxns = dp("fxns", (NS, BK))
    fict = dp("fict", (BK, NS))
    fist = dp("fist", (BK, NS))
    finst = dp("finst", (BK, NS))
    gt = dp("gt", (BK, KYH, KZB))                  # G/det for own ky half
    idn = dp("idn", (128, 128))
    outp = nc.declare_dram_parameter("out", [128, NS // XP], F32, isOutput=True)
    mult = mybir.AluOpType.mult
    add = mybir.AluOpType.add

    with tile.TileContext(nc) as tc:
        with (
            tc.tile_pool(name="constp", bufs=1) as constp,
            tc.tile_pool(name="iop", bufs=8) as iop,
            tc.tile_pool(name="psp", bufs=4, space="PSUM") as psp,
        ):
            # issue the first spread loads before the constants so P12 can
            # start the moment data lands
            spt_pre = []
            for g4 in range(2):
                spt = iop.tile([2 * C * ST, NPAIR, SW], BF16, tag="spt")
                if g4 == 0:
                    nc.sync.dma_start(spt[:, 0:NPAIR // 2], spx[g4, :, 0:NPAIR // 2])
                    nc.sync.dma_start(spt[:, NPAIR // 2:], spx[g4, :, NPAIR // 2:])
                else:
                    nc.sync.dma_start(spt[:], spx[g4])
                spt_pre.append(spt)
            FXC = constp.tile([128, 2, BK], BF16)
            FXS = constp.tile([128, 2, BK], BF16)
            FXNS = constp.tile([128, 2, BK], BF16)
            for ch in range(2):
                nc.sync.dma_start(FXC[:, ch], fxc[128 * ch:128 * (ch + 1), :])
                nc.sync.dma_start(FXS[:, ch], fxs[128 * ch:128 * (ch + 1), :])
                nc.sync.dma_start(FXNS[:, ch], fxns[128 * ch:128 * (ch + 1), :])
            FICT = constp.tile([BK, NS], BF16)
            FIST = constp.tile([BK, NS], BF16)
            FINST = constp.tile([BK, NS], BF16)
            nc.sync.dma_start(FICT[:], fict[:])
            nc.sync.dma_start(FIST[:], fist[:])
            nc.sync.dma_start(FINST[:], finst[:])
            GT = constp.tile([BK, KYH, KZB], BF16)
            nc.sync.dma_start(GT[:], gt[:])
            IDN = constp.tile([128, 128], BF16)
            nc.sync.dma_start(IDN[:], idn[:])
            OUT = constp.tile([128, NS // XP], F32)

            # SBUF-resident spectral cubes (bf16)
            CB2 = constp.tile([BK, NS, KYH], BF16)          # (kzri, x, ky)
            CB3 = constp.tile([128, 2, KYH, KRIB], BF16)    # (x, xch, ky, kzri)
            CB4 = constp.tile([BK, KYH, KRIB], BF16)        # (kx, ky, kzri)
            CB5 = constp.tile([128, 2, KYH, KRIB], BF16)    # (x, xch, ky, kzri)
            CB6 = constp.tile([BK, KYH, NS], BF16)          # (kzri, ky, x)

            # ---------------- P12: spread (analytic y,z) ----------------
            # ST x cells share one matmul: lhsT stacks their [L1;L2] blocks on
            # the contract dim; rhs is block-diagonal so outputs stay separate
            for g4 in range(NS // XG):
                if g4 < 2:
                    spt = spt_pre[g4]
                else:
                    spt = iop.tile([2 * C * ST, NPAIR, SW], BF16, tag="spt")
                    nc.sync.dma_start(spt[:], spx[g4])
                ps = psp.tile([BK, NPAIR, ST, KYH], F32, tag="A")
                for p in range(NPAIR):
                    nc.tensor.matmul(ps[:, p], spt[:, p, 0:64],
                                     spt[:, p, 64:SW], start=True, stop=True)
                x0 = g4 * XG
                if g4 % 2 == 0:
                    nc.scalar.copy(CB2[:, x0:x0 + XG, :], ps[:])
                else:
                    nc.vector.tensor_copy(CB2[:, x0:x0 + XG, :], ps[:])

            # ---------------- T1: (kzri, x) -> (x, kzri) ----------------
            for ky0 in range(0, KYH, 8):
                for xch in range(2):
                    xsl = slice(128 * xch, 128 * (xch + 1))
                    pst = psp.tile([128, 8, BK], BF16, tag="B")
                    for i in range(8):
                        nc.tensor.transpose(pst[:, i, :], CB2[:, xsl, ky0 + i],
                                            IDN[0:BK, 0:BK])
                    if (ky0 // 8 + xch) % 2 == 0:
                        nc.scalar.copy(CB3[:, xch, ky0:ky0 + 8, :], pst[:])
                    else:
                        nc.vector.tensor_copy(CB3[:, xch, ky0:ky0 + 8, :], pst[:])

            # ---------------- P3: x-DFT (banded) + G ---------------------
            for kyg in range(0, KYH, 16):
                ksl = slice(kyg, kyg + 16)
                pxr = psp.tile([BK, 16, KZB], F32, tag="A")
                pxi = psp.tile([BK, 16, KZB], F32, tag="B")
                # XR = Fxc@CR + Fxs@CI ; XI = Fxc@CI - Fxs@CR
                nc.tensor.matmul(pxr[:], FXC[:, 0], CB3[:, 0, ksl, 0:KZB], start=True, stop=False)
                nc.tensor.matmul(pxr[:], FXC[:, 1], CB3[:, 1, ksl, 0:KZB], start=False, stop=False)
                nc.tensor.matmul(pxr[:], FXS[:, 0], CB3[:, 0, ksl, KZB:KRIB], start=False, stop=False)
                nc.tensor.matmul(pxr[:], FXS[:, 1], CB3[:, 1, ksl, KZB:KRIB], start=False, stop=True)
                nc.tensor.matmul(pxi[:], FXC[:, 0], CB3[:, 0, ksl, KZB:KRIB], start=True, stop=False)
                nc.tensor.matmul(pxi[:], FXC[:, 1], CB3[:, 1, ksl, KZB:KRIB], start=False, stop=False)
                nc.tensor.matmul(pxi[:], FXNS[:, 0], CB3[:, 0, ksl, 0:KZB], start=False, stop=False)
                nc.tensor.matmul(pxi[:], FXNS[:, 1], CB3[:, 1, ksl, 0:KZB], start=False, stop=True)
                nc.vector.tensor_tensor(CB4[:, ksl, 0:KZB], pxr[:], GT[:, ksl, :], op=mult)
                nc.vector.tensor_tensor(CB4[:, ksl, KZB:KRIB], pxi[:], GT[:, ksl, :], op=mult)

            # ---------------- P4: inverse x-DFT --------------------------
            for kyg in range(0, KYH, 16):
                ksl = slice(kyg, kyg + 16)
                for xch in range(2):
                    xsl = slice(128 * xch, 128 * (xch + 1))
                    pvr = psp.tile([128, 16, KZB], F32, tag="A")
                    pvi = psp.tile([128, 16, KZB], F32, tag="B")
                    # VR = Fic@XR - Fis@XI ; VI = Fis@XR + Fic@XI
                    nc.tensor.matmul(pvr[:], FICT[:, xsl], CB4[:, ksl, 0:KZB], start=True, stop=False)
                    nc.tensor.matmul(pvr[:], FINST[:, xsl], CB4[:, ksl, KZB:KRIB], start=False, stop=True)
                    nc.tensor.matmul(pvi[:], FIST[:, xsl], CB4[:, ksl, 0:KZB], start=True, stop=False)
                    nc.tensor.matmul(pvi[:], FICT[:, xsl], CB4[:, ksl, KZB:KRIB], start=False, stop=True)
                    nc.scalar.copy(CB5[:, xch, ksl, 0:KZB], pvr[:])
                    nc.vector.tensor_copy(CB5[:, xch, ksl, KZB:KRIB], pvi[:])

            # ---------------- T2: (x, kzri) -> (kzri, x) ----------------
            for ky0 in range(0, KYH, 8):
                for xch in range(2):
                    xsl = slice(128 * xch, 128 * (xch + 1))
                    pst = psp.tile([BK, 8, 128], BF16, tag="B")
                    for i in range(8):
                        nc.tensor.transpose(pst[:, i, :], CB5[:, xch, ky0 + i, :], IDN[:])
                    if (ky0 // 8 + xch) % 2 == 0:
                        nc.scalar.copy(CB6[:, ky0:ky0 + 8, xsl], pst[:])
                    else:
                        nc.vector.tensor_copy(CB6[:, ky0:ky0 + 8, xsl], pst[:])

            # ---------------- P56: gather (analytic y,z) -----------------
            # ps6[xi*C+s, gi, comp, ky, xj] = (M_comp of xi)^T V(xj); host-
            # baked gy mask keeps only xj == xi.  scalar evacuates PSUM,
            # gpsimd does the SyPack multiply, vector reduces.
            for q in range(G2 // 2):
                scr = iop.tile([128, 2, 2, 2, KYH, XP], BF16, tag="scr")
                for half in range(2):
                    g2 = 2 * q + half
                    gmt = iop.tile([BK, 2, 2, XP * C], BF16, tag="gmt")
                    nc.sync.dma_start(gmt[:], gx[g2])
                    ps6 = psp.tile([128, 2, 2, KYH, XP], F32, tag="A")
                    for gi in range(2):
                        g = 2 * g2 + gi
                        for comp in range(2):
                            nc.tensor.matmul(ps6[:, gi, comp], gmt[:, gi, comp, :],
                                             CB6[:, :, g * XP:(g + 1) * XP],
                                             start=True, stop=True)
                    gyt = iop.tile([128, 2, 2, KYH, XP], BF16, tag="gyt")
                    nc.sync.dma_start(gyt[:], gy[g2])
                    if g2 % 3 != 2:
                        s6 = iop.tile([128, 2, 2, KYH, XP], F32, tag="s6")
                        nc.scalar.copy(s6[:], ps6[:])
                        nc.gpsimd.tensor_tensor(scr[:, half], s6[:], gyt[:], op=mult)
                    else:
                        nc.vector.tensor_tensor(scr[:, half], ps6[:], gyt[:], op=mult)
                nc.vector.tensor_reduce(OUT[:, 4 * q:4 * q + 4], scr[:],
                                        axis=mybir.AxisListType.XYZ, op=add)
                if q % 4 == 3:
                    nc.sync.dma_start(outp[:, 16 * (q // 4):16 * (q // 4) + 16],
                                      OUT[:, 16 * (q // 4):16 * (q // 4) + 16])
    nc.compile()
    return nc


def host_prep(cell, positions, charges):
    NA = positions.shape[0]
    NSP = charges.shape[1]
    cell = np.asarray(cell, dtype=np.float64)
    positions = np.asarray(positions, dtype=np.float64)
    charges = np.asarray(charges, dtype=np.float64)

    inv_cell = np.linalg.inv(cell)
    pos_rel = NS * (positions @ inv_cell)
    idx0 = np.floor(pos_rel)
    t = pos_rel - (idx0 + 0.5)
    t2 = t * t
    t3 = t2 * t
    w = np.stack([
        (1 - 6 * t + 12 * t2 - 8 * t3) / 48,
        (23 - 30 * t - 12 * t2 + 24 * t3) / 48,
        (23 + 30 * t - 12 * t2 - 24 * t3) / 48,
        (1 + 6 * t + 12 * t2 + 8 * t3) / 48,
    ])  # (4, NA, 3)
    offs = np.arange(-1, 3)
    idx = (idx0.astype(np.int64)[None] + offs[:, None, None]) % NS  # (4, NA, 3)

    nb = np.r_[0:BK // 2, -BK // 2:0].astype(np.float64)   # band freqs, fft order
    kzb = np.arange(KZB, dtype=np.float64)
    wkz = np.where(kzb == 0, 1.0, 2.0)

    # per-atom structure factors over the band
    Sy = np.zeros((NA, BK), dtype=np.complex128)
    Sz = np.zeros((NA, KZB), dtype=np.complex128)
    for j in range(4):
        Sy += w[j, :, 1:2] * np.exp(-2j * np.pi * np.outer(idx[j, :, 1], nb) / NS)
        Sz += w[j, :, 2:3] * np.exp(-2j * np.pi * np.outer(idx[j, :, 2], kzb) / NS)

    # bins over x cells
    entries = [[] for _ in range(NS)]
    for j in range(4):
        for n in range(NA):
            entries[idx[j, n, 0]].append((n, w[j, n, 0]))
    # snap bin capacity to {32, 64, 128} so per-x partition offsets (C*xi)
    # land on legal PE tile positions, and XP divides NS
    raw = max(len(e) for e in entries)
    assert raw <= 128, f"x-bin overflow: {raw}"
    C = 32 if raw <= 32 else (64 if raw <= 64 else 128)
    XP = 128 // C
    atom_of = np.zeros((NS, C), dtype=np.int64)
    wx_of = np.zeros((NS, C))
    valid = np.zeros((NS, C), dtype=bool)
    for x in range(NS):
        for s, (n, wx) in enumerate(entries[x]):
            atom_of[x, s] = n
            wx_of[x, s] = wx
            valid[x, s] = True

    SyB = Sy[atom_of]                    # (NS, C, BK)
    SzB = Sz[atom_of]                    # (NS, C, KZB)
    wv = (wx_of * valid)[..., None]

    # spread inputs per channel & ky-half, stacked on the contract dim:
    # rows [0:C] = (L1 | R1), rows [C:2C] = (L2 | R2), so one matmul per x
    SPX = {}
    for ch in range(NSP):
        a = (charges[atom_of, ch] * wx_of * valid)[..., None]
        L1 = np.concatenate([SzB.real, SzB.imag], -1) * a
        L2 = np.concatenate([-SzB.imag, SzB.real], -1) * a
        for h in range(2):
            ksl = slice(h * KYH, (h + 1) * KYH)
            Lst = np.concatenate([L1, L2], 1)                     # (NS,2C,64)
            Rst = np.concatenate(
                [SyB.real[:, :, ksl], SyB.imag[:, :, ksl]], 1)    # (NS,2C,32)
            ST = max(1, 128 // (2 * C))
            XG = 16
            SW = 64 + 32 * ST
            A = np.zeros((NS // ST, 2 * C * ST, SW))
            for s in range(ST):
                rows = slice(s * 2 * C, (s + 1) * 2 * C)
                A[:, rows, 0:64] = Lst[s::ST]
                A[:, rows, 64 + 32 * s:96 + 32 * s] = Rst[s::ST]
            SPX[(ch, h)] = np.ascontiguousarray(
                A.reshape(NS // XG, XG // ST, 2 * C * ST, SW)
                .transpose(0, 2, 1, 3)).astype(BFNP)

    # gather matrices (channel-independent): gx[g, kzri, comp, xi*C+s]
    M1 = (np.concatenate([SzB.real * wkz, SzB.imag * wkz], -1) * wv)   # (NS,C,64)
    M2 = (np.concatenate([-SzB.imag * wkz, SzB.real * wkz], -1) * wv)
    GX = np.ascontiguousarray(
        np.stack([M1, M2], 1)                      # (NS, 2, C, 64)
        .reshape(NS // (2 * XP), 2, XP, 2, C, BK).transpose(0, 5, 1, 3, 2, 4)
        .reshape(NS // (2 * XP), BK, 2, 2, XP * C)).astype(BFNP)

    # diag-masked SyPack: gy[g2, xi*C+s, gi, comp, xj, ky] = Sy_comp(s@x)*[xj==xi]
    GY = {}
    for h in range(2):
        ksl = slice(h * KYH, (h + 1) * KYH)
        syp = np.stack([SyB.real[:, :, ksl], SyB.imag[:, :, ksl]], 2) * valid[..., None, None]
        syg = syp.reshape(NS // (2 * XP), 2, XP, C, 2, KYH)   # (G2, gi, xi, s, comp, ky)
        g7 = np.zeros((NS // (2 * XP), 2, XP, C, 2, KYH, XP))
        for xi in range(XP):
            g7[:, :, xi, :, :, :, xi] = syg[:, :, xi]
        GY[h] = np.ascontiguousarray(
            g7.transpose(0, 2, 3, 1, 4, 5, 6)
            .reshape(NS // (2 * XP), XP * C, 2, 2, KYH, XP)).astype(BFNP)

    # banded Coulomb kernel / det
    recip = 2 * np.pi * inv_cell.T
    kxg, kyg, kzg = np.meshgrid(nb, nb, kzb, indexing="ij")
    kvec = kxg[..., None] * recip[0] + kyg[..., None] * recip[1] + kzg[..., None] * recip[2]
    ksq = np.sum(kvec * kvec, axis=-1)
    G = np.where(ksq == 0, 0.0,
                 4 * np.pi * np.exp(-0.5 * SMEARING**2 * ksq) / np.where(ksq == 0, 1.0, ksq))
    G = G / np.abs(np.linalg.det(cell))
    GTS = {h: np.ascontiguousarray(G[:, h * KYH:(h + 1) * KYH, :]).astype(BFNP)
           for h in range(2)}

    # band DFT matrices
    th = 2 * np.pi * np.outer(np.arange(NS), nb) / NS
    Fxc = np.cos(th).astype(BFNP)
    Fxs = np.sin(th).astype(BFNP)
    return dict(C=C, XP=XP, NSP=NSP, NA=NA, atom_of=atom_of, valid=valid,
                SPX=SPX, GX=GX, GY=GY, GTS=GTS,
                Fxc=Fxc, Fxs=Fxs, Fxns=(-Fxs).astype(BFNP),
                Fict=np.ascontiguousarray(Fxc.T), Fist=np.ascontiguousarray(Fxs.T),
                Finst=np.ascontiguousarray((-Fxs).astype(BFNP).T),
                idn=np.eye(128, dtype=BFNP))


def _run(cell, positions, charges, trace=False):
    prep = host_prep(cell, positions, charges)
    C = prep["C"]
    XP = prep["XP"]
    if C not in _cache:
        _cache[C] = build_program(C)
    nc = _cache[C]

    in_maps = []
    for core in range(N_CORES):
        ch, h = divmod(core, 2)
        in_maps.append({
            "spx": prep["SPX"][(ch, h)],
            "gx": prep["GX"],
            "gy": prep["GY"][h],
            "fxc": prep["Fxc"], "fxs": prep["Fxs"], "fxns": prep["Fxns"],
            "fict": prep["Fict"], "fist": prep["Fist"], "finst": prep["Finst"],
            "gt": prep["GTS"][h],
            "idn": prep["idn"],
        })
    res = run_bass_kernel_spmd(nc, in_maps, list(range(N_CORES)), trace=trace)

    NA, NSP = prep["NA"], prep["NSP"]
    pot = np.zeros((NA, NSP), dtype=np.float64)
    valid = prep["valid"]
    atom_flat = prep["atom_of"][valid]
    for core in range(N_CORES):
        ch, h = divmod(core, 2)
        out = res.results[core]["out"]          # (128, NS//XP): row xi*C+s, col g
        out_cs = out.reshape(XP, C, NS // XP).transpose(2, 0, 1).reshape(NS, C)
        np.add.at(pot[:, ch], atom_flat, out_cs[valid])
    return pot.astype(np.float32), res


def kernel(cell, positions, charges):
    pot, _ = _run(cell, positions, charges, trace=False)
    return pot


# revision 28
# speedup vs baseline: 1.0274x; 1.0274x over previous
"""MeshPotential (P3M-style) Trainium2 kernel — banded-spectrum version.

Key physics: with atomic smearing 0.4 the k-space kernel G ~ exp(-0.0079 n^2)
is < 1e-7 outside integer frequencies |n| <= 32.  So only a 64 x 64 x 32
band of the 256^3 rfft spectrum matters (verified: truncation rel err 4e-6).

Per-core (8 cores SPMD, core = (channel, ky-half)) the pipeline is fully
analytic in y and z (per-atom structure factors, host-precomputed) and a
dense 256-point DFT in x only:

  P12  spread     : R(kzri, ky | x) = sum_slots a*wx*[SzR|SzI] (x) Sy
  T1   transpose  : [kzri, x] -> [x, kzri] blocks (PE transpose)
  P3   x-DFT + G  : X(kx, ky, kzri) = F_x R ;  X *= G   (banded kx: 64)
  P4   x-inverse  : V(x, ky, kzri)  = F_x^H X
  T2   transpose  : [x, kzri] -> [kzri, x] blocks
  P56  gather     : U(slot, ky) = M^T V ; pot(slot|x) = sum_ky SyPack * U

All matmul operands are bf16 (PSUM accumulates fp32).  Whole spectral cube
lives in SBUF (~5 MB); no DRAM round trips.  Host folds bin slots back to
atoms and sums the two ky-half cores per channel.
"""

import os

import numpy as np
import ml_dtypes

import concourse.bass as bass
import concourse.mybir as mybir
import concourse.tile as tile
from concourse import bacc
from concourse.bass_utils import run_bass_kernel_spmd

F32 = mybir.dt.float32
BF16 = mybir.dt.bfloat16
BFNP = ml_dtypes.bfloat16

NS = 256
BK = 64            # kx / ky band size (freqs 0..31, -32..-1)
KZB = 32           # kz band size (0..31)
KRIB = 2 * KZB     # [Re | Im] packed kz
KYH = 32           # ky values per core (half of band)
N_CORES = 8
SMEARING = 0.4
BOX_REF = None     # general cell handled via inv_cell in host_prep

_cache = {}


def build_program(C):
    XP = 128 // C                  # x cells per spread/gather sub-group
    G2 = NS // (2 * XP)            # gather batches (2 groups each)
    nc = bacc.Bacc(None, target_bir_lowering=False, debug=False)
    dp = lambda name, shape, dt=BF16: nc.declare_dram_parameter(
        name, list(shape), dt, isOutput=False)
    ST = max(1, 128 // (2 * C))    # x cells stacked per spread matmul
    XG = 16                        # x cells per spread DMA group
    NPAIR = XG // ST
    SW = 64 + 32 * ST
    spx = dp("spx", (NS // XG, 2 * C * ST, NPAIR, SW))   # [L-stack | blockdiag R]
    gx = dp("gx", (G2, BK, 2, 2, XP * C))          # [M1-all-xi | M2-all-xi]
    gy = dp("gy", (G2, 128, 2, 2, KYH, XP))        # diag-masked SyPack
    fxc = dp("fxc", (NS, BK))
    fxs = dp("fxs", (NS, BK))
    fxns = dp("fxns", (NS, BK))
    fict = dp("fict", (BK, NS))
    fist = dp("fist", (BK, NS))
    finst = dp("finst", (BK, NS))
    gt = dp("gt", (BK, KYH, KZB))                  # G/det for own ky half
    idn = dp("idn", (128, 128))
    outp = nc.declare_dram_parameter("out", [128, NS // XP], F32, isOutput=True)
    mult = mybir.AluOpType.mult
    add = mybir.AluOpType.add

    with tile.TileContext(nc) as tc:
        with (
            tc.tile_pool(name="constp", bufs=1) as constp,
            tc.tile_pool(name="iop", bufs=8) as iop,
            tc.tile_pool(name="psp", bufs=4, space="PSUM") as psp,
        ):
            # issue the first spread loads before the constants so P12 can
            # start the moment data lands
            spt_pre = []
            for g4 in range(2):
                spt = iop.tile([2 * C * ST, NPAIR, SW], BF16, tag="spt")
                nc.sync.dma_start(spt[:], spx[g4])
                spt_pre.append(spt)
            FXC = constp.tile([128, 2, BK], BF16)
            FXS = constp.tile([128, 2, BK], BF16)
            FXNS = constp.tile([128, 2, BK], BF16)
            for ch in range(2):
                nc.sync.dma_start(FXC[:, ch], fxc[128 * ch:128 * (ch + 1), :])
                nc.sync.dma_start(FXS[:, ch], fxs[128 * ch:128 * (ch + 1), :])
                nc.sync.dma_start(FXNS[:, ch], fxns[128 * ch:128 * (ch + 1), :])
            FICT = constp.tile([BK, NS], BF16)
            FIST = constp.tile([BK, NS], BF16)
            FINST = constp.tile([BK, NS], BF16)
            nc.sync.dma_start(FICT[:], fict[:])
            nc.sync.dma_start(FIST[:], fist[:])
            nc.sync.dma_start(FINST[:], finst[:])
            GT = constp.tile([BK, KYH, KZB], BF16)
            nc.sync.dma_start(GT[:], gt[:])
            IDN = constp.tile([128, 128], BF16)
            nc.sync.dma_start(IDN[:], idn[:])
            OUT = constp.tile([128, NS // XP], F32)

            # SBUF-resident spectral cubes (bf16)
            CB2 = constp.tile([BK, NS, KYH], BF16)          # (kzri, x, ky)
            CB3 = constp.tile([128, 2, KYH, KRIB], BF16)    # (x, xch, ky, kzri)
            CB4 = constp.tile([BK, KYH, KRIB], BF16)        # (kx, ky, kzri)
            CB5 = constp.tile([128, 2, KYH, KRIB], BF16)    # (x, xch, ky, kzri)
            CB6 = constp.tile([BK, KYH, NS], BF16)          # (kzri, ky, x)

            # ---------------- P12: spread (analytic y,z) ----------------
            # ST x cells share one matmul: lhsT stacks their [L1;L2] blocks on
            # the contract dim; rhs is block-diagonal so outputs stay separate
            for g4 in range(NS // XG):
                if g4 < 2:
                    spt = spt_pre[g4]
                else:
                    spt = iop.tile([2 * C * ST, NPAIR, SW], BF16, tag="spt")
                    nc.sync.dma_start(spt[:], spx[g4])
                ps = psp.tile([BK, NPAIR, ST, KYH], F32, tag="A")
                for p in range(NPAIR):
                    nc.tensor.matmul(ps[:, p], spt[:, p, 0:64],
                                     spt[:, p, 64:SW], start=True, stop=True)
                x0 = g4 * XG
                if g4 % 2 == 0:
                    nc.scalar.copy(CB2[:, x0:x0 + XG, :], ps[:])
                else:
                    nc.vector.tensor_copy(CB2[:, x0:x0 + XG, :], ps[:])

            # ---------------- T1: (kzri, x) -> (x, kzri) ----------------
            for ky0 in range(0, KYH, 8):
                for xch in range(2):
                    xsl = slice(128 * xch, 128 * (xch + 1))
                    pst = psp.tile([128, 8, BK], BF16, tag="B")
                    for i in range(8):
                        nc.tensor.transpose(pst[:, i, :], CB2[:, xsl, ky0 + i],
                                            IDN[0:BK, 0:BK])
                    if (ky0 // 8 + xch) % 2 == 0:
                        nc.scalar.copy(CB3[:, xch, ky0:ky0 + 8, :], pst[:])
                    else:
                        nc.vector.tensor_copy(CB3[:, xch, ky0:ky0 + 8, :], pst[:])

            # ---------------- P3: x-DFT (banded) + G ---------------------
            for kyg in range(0, KYH, 16):
                ksl = slice(kyg, kyg + 16)
                pxr = psp.tile([BK, 16, KZB], F32, tag="A")
                pxi = psp.tile([BK, 16, KZB], F32, tag="B")
                # XR = Fxc@CR + Fxs@CI ; XI = Fxc@CI - Fxs@CR
                nc.tensor.matmul(pxr[:], FXC[:, 0], CB3[:, 0, ksl, 0:KZB], start=True, stop=False)
                nc.tensor.matmul(pxr[:], FXC[:, 1], CB3[:, 1, ksl, 0:KZB], start=False, stop=False)
                nc.tensor.matmul(pxr[:], FXS[:, 0], CB3[:, 0, ksl, KZB:KRIB], start=False, stop=False)
                nc.tensor.matmul(pxr[:], FXS[:, 1], CB3[:, 1, ksl, KZB:KRIB], start=False, stop=True)
                nc.tensor.matmul(pxi[:], FXC[:, 0], CB3[:, 0, ksl, KZB:KRIB], start=True, stop=False)
                nc.tensor.matmul(pxi[:], FXC[:, 1], CB3[:, 1, ksl, KZB:KRIB], start=False, stop=False)
                nc.tensor.matmul(pxi[:], FXNS[:, 0], CB3[:, 0, ksl, 0:KZB], start=False, stop=False)
                nc.tensor.matmul(pxi[:], FXNS[:, 1], CB3[:, 1, ksl, 0:KZB], start=False, stop=True)
                nc.vector.tensor_tensor(CB4[:, ksl, 0:KZB], pxr[:], GT[:, ksl, :], op=mult)
                nc.vector.tensor_tensor(CB4[:, ksl, KZB:KRIB], pxi[:], GT[:, ksl, :], op=mult)

            # ---------------- P4: inverse x-DFT --------------------------
            for kyg in range(0, KYH, 16):
                ksl = slice(kyg, kyg + 16)
                for xch in range(2):
                    xsl = slice(128 * xch, 128 * (xch + 1))
                    pvr = psp.tile([128, 16, KZB], F32, tag="A")
                    pvi = psp.tile([128, 16, KZB], F32, tag="B")
                    # VR = Fic@XR - Fis@XI ; VI = Fis@XR + Fic@XI
                    nc.tensor.matmul(pvr[:], FICT[:, xsl], CB4[:, ksl, 0:KZB], start=True, stop=False)
                    nc.tensor.matmul(pvr[:], FINST[:, xsl], CB4[:, ksl, KZB:KRIB], start=False, stop=True)
                    nc.tensor.matmul(pvi[:], FIST[:, xsl], CB4[:, ksl, 0:KZB], start=True, stop=False)
                    nc.tensor.matmul(pvi[:], FICT[:, xsl], CB4[:, ksl, KZB:KRIB], start=False, stop=True)
                    nc.scalar.copy(CB5[:, xch, ksl, 0:KZB], pvr[:])
                    nc.vector.tensor_copy(CB5[:, xch, ksl, KZB:KRIB], pvi[:])

            # ---------------- T2: (x, kzri) -> (kzri, x) ----------------
            for ky0 in range(0, KYH, 8):
                for xch in range(2):
                    xsl = slice(128 * xch, 128 * (xch + 1))
                    pst = psp.tile([BK, 8, 128], BF16, tag="B")
                    for i in range(8):
                        nc.tensor.transpose(pst[:, i, :], CB5[:, xch, ky0 + i, :], IDN[:])
                    if (ky0 // 8 + xch) % 2 == 0:
                        nc.scalar.copy(CB6[:, ky0:ky0 + 8, xsl], pst[:])
                    else:
                        nc.vector.tensor_copy(CB6[:, ky0:ky0 + 8, xsl], pst[:])

            # ---------------- P56: gather (analytic y,z) -----------------
            # ps6[xi*C+s, gi, comp, ky, xj] = (M_comp of xi)^T V(xj); host-
            # baked gy mask keeps only xj == xi.  scalar evacuates PSUM,
            # gpsimd does the SyPack multiply, vector reduces.
            for q in range(G2 // 2):
                scr = iop.tile([128, 2, 2, 2, KYH, XP], F32, tag="scr")
                for half in range(2):
                    g2 = 2 * q + half
                    gmt = iop.tile([BK, 2, 2, XP * C], BF16, tag="gmt")
                    nc.sync.dma_start(gmt[:], gx[g2])
                    ps6 = psp.tile([128, 2, 2, KYH, XP], F32, tag="A")
                    for gi in range(2):
                        g = 2 * g2 + gi
                        for comp in range(2):
                            nc.tensor.matmul(ps6[:, gi, comp], gmt[:, gi, comp, :],
                                             CB6[:, :, g * XP:(g + 1) * XP],
                                             start=True, stop=True)
                    gyt = iop.tile([128, 2, 2, KYH, XP], BF16, tag="gyt")
                    nc.sync.dma_start(gyt[:], gy[g2])
                    if g2 % 2 == 0:
                        s6 = iop.tile([128, 2, 2, KYH, XP], F32, tag="s6")
                        nc.scalar.copy(s6[:], ps6[:])
                        nc.gpsimd.tensor_tensor(scr[:, half], s6[:], gyt[:], op=mult)
                    else:
                        nc.vector.tensor_tensor(scr[:, half], ps6[:], gyt[:], op=mult)
                nc.vector.tensor_reduce(OUT[:, 4 * q:4 * q + 4], scr[:],
                                        axis=mybir.AxisListType.XYZ, op=add)
                if q % 4 == 3:
                    nc.sync.dma_start(outp[:, 16 * (q // 4):16 * (q // 4) + 16],
                                      OUT[:, 16 * (q // 4):16 * (q // 4) + 16])
    nc.compile()
    return nc


def host_prep(cell, positions, charges):
    NA = positions.shape[0]
    NSP = charges.shape[1]
    cell = np.asarray(cell, dtype=np.float64)
    positions = np.asarray(positions, dtype=np.float64)
    charges = np.asarray(charges, dtype=np.float64)

    inv_cell = np.linalg.inv(cell)
    pos_rel = NS * (positions @ inv_cell)
    idx0 = np.floor(pos_rel)
    t = pos_rel - (idx0 + 0.5)
    t2 = t * t
    t3 = t2 * t
    w = np.stack([
        (1 - 6 * t + 12 * t2 - 8 * t3) / 48,
        (23 - 30 * t - 12 * t2 + 24 * t3) / 48,
        (23 + 30 * t - 12 * t2 - 24 * t3) / 48,
        (1 + 6 * t + 12 * t2 + 8 * t3) / 48,
    ])  # (4, NA, 3)
    offs = np.arange(-1, 3)
    idx = (idx0.astype(np.int64)[None] + offs[:, None, None]) % NS  # (4, NA, 3)

    nb = np.r_[0:BK // 2, -BK // 2:0].astype(np.float64)   # band freqs, fft order
    kzb = np.arange(KZB, dtype=np.float64)
    wkz = np.where(kzb == 0, 1.0, 2.0)

    # per-atom structure factors over the band
    Sy = np.zeros((NA, BK), dtype=np.complex128)
    Sz = np.zeros((NA, KZB), dtype=np.complex128)
    for j in range(4):
        Sy += w[j, :, 1:2] * np.exp(-2j * np.pi * np.outer(idx[j, :, 1], nb) / NS)
        Sz += w[j, :, 2:3] * np.exp(-2j * np.pi * np.outer(idx[j, :, 2], kzb) / NS)

    # bins over x cells
    entries = [[] for _ in range(NS)]
    for j in range(4):
        for n in range(NA):
            entries[idx[j, n, 0]].append((n, w[j, n, 0]))
    # snap bin capacity to {32, 64, 128} so per-x partition offsets (C*xi)
    # land on legal PE tile positions, and XP divides NS
    raw = max(len(e) for e in entries)
    assert raw <= 128, f"x-bin overflow: {raw}"
    C = 32 if raw <= 32 else (64 if raw <= 64 else 128)
    XP = 128 // C
    atom_of = np.zeros((NS, C), dtype=np.int64)
    wx_of = np.zeros((NS, C))
    valid = np.zeros((NS, C), dtype=bool)
    for x in range(NS):
        for s, (n, wx) in enumerate(entries[x]):
            atom_of[x, s] = n
            wx_of[x, s] = wx
            valid[x, s] = True

    SyB = Sy[atom_of]                    # (NS, C, BK)
    SzB = Sz[atom_of]                    # (NS, C, KZB)
    wv = (wx_of * valid)[..., None]

    # spread inputs per channel & ky-half, stacked on the contract dim:
    # rows [0:C] = (L1 | R1), rows [C:2C] = (L2 | R2), so one matmul per x
    SPX = {}
    for ch in range(NSP):
        a = (charges[atom_of, ch] * wx_of * valid)[..., None]
        L1 = np.concatenate([SzB.real, SzB.imag], -1) * a
        L2 = np.concatenate([-SzB.imag, SzB.real], -1) * a
        for h in range(2):
            ksl = slice(h * KYH, (h + 1) * KYH)
            Lst = np.concatenate([L1, L2], 1)                     # (NS,2C,64)
            Rst = np.concatenate(
                [SyB.real[:, :, ksl], SyB.imag[:, :, ksl]], 1)    # (NS,2C,32)
            ST = max(1, 128 // (2 * C))
            XG = 16
            SW = 64 + 32 * ST
            A = np.zeros((NS // ST, 2 * C * ST, SW))
            for s in range(ST):
                rows = slice(s * 2 * C, (s + 1) * 2 * C)
                A[:, rows, 0:64] = Lst[s::ST]
                A[:, rows, 64 + 32 * s:96 + 32 * s] = Rst[s::ST]
            SPX[(ch, h)] = np.ascontiguousarray(
                A.reshape(NS // XG, XG // ST, 2 * C * ST, SW)
                .transpose(0, 2, 1, 3)).astype(BFNP)

    # gather matrices (channel-independent): gx[g, kzri, comp, xi*C+s]
    M1 = (np.concatenate([SzB.real * wkz, SzB.imag * wkz], -1) * wv)   # (NS,C,64)
    M2 = (np.concatenate([-SzB.imag * wkz, SzB.real * wkz], -1) * wv)
    GX = np.ascontiguousarray(
        np.stack([M1, M2], 1)                      # (NS, 2, C, 64)
        .reshape(NS // (2 * XP), 2, XP, 2, C, BK).transpose(0, 5, 1, 3, 2, 4)
        .reshape(NS // (2 * XP), BK, 2, 2, XP * C)).astype(BFNP)

    # diag-masked SyPack: gy[g2, xi*C+s, gi, comp, xj, ky] = Sy_comp(s@x)*[xj==xi]
    GY = {}
    for h in range(2):
        ksl = slice(h * KYH, (h + 1) * KYH)
        syp = np.stack([SyB.real[:, :, ksl], SyB.imag[:, :, ksl]], 2) * valid[..., None, None]
        syg = syp.reshape(NS // (2 * XP), 2, XP, C, 2, KYH)   # (G2, gi, xi, s, comp, ky)
        g7 = np.zeros((NS // (2 * XP), 2, XP, C, 2, KYH, XP))
        for xi in range(XP):
            g7[:, :, xi, :, :, :, xi] = syg[:, :, xi]
        GY[h] = np.ascontiguousarray(
            g7.transpose(0, 2, 3, 1, 4, 5, 6)
            .reshape(NS // (2 * XP), XP * C, 2, 2, KYH, XP)).astype(BFNP)

    # banded Coulomb kernel / det
    recip = 2 * np.pi * inv_cell.T
    kxg, kyg, kzg = np.meshgrid(nb, nb, kzb, indexing="ij")
    kvec = kxg[..., None] * recip[0] + kyg[..., None] * recip[1] + kzg[..., None] * recip[2]
    ksq = np.sum(kvec * kvec, axis=-1)
    G = np.where(ksq == 0, 0.0,
                 4 * np.pi * np.exp(-0.5 * SMEARING**2 * ksq) / np.where(ksq == 0, 1.0, ksq))
    G = G / np.abs(np.linalg.det(cell))
    GTS = {h: np.ascontiguousarray(G[:, h * KYH:(h + 1) * KYH, :]).astype(BFNP)
           for h in range(2)}

    # band DFT matrices
    th = 2 * np.pi * np.outer(np.arange(NS), nb) / NS
    Fxc = np.cos(th).astype(BFNP)
    Fxs = np.sin(th).astype(BFNP)
    return dict(C=C, XP=XP, NSP=NSP, NA=NA, atom_of=atom_of, valid=valid,
                SPX=SPX, GX=GX, GY=GY, GTS=GTS,
                Fxc=Fxc, Fxs=Fxs, Fxns=(-Fxs).astype(BFNP),
                Fict=np.ascontiguousarray(Fxc.T), Fist=np.ascontiguousarray(Fxs.T),
                Finst=np.ascontiguousarray((-Fxs).astype(BFNP).T),
                idn=np.eye(128, dtype=BFNP))


def _run(cell, positions, charges, trace=False):
    prep = host_prep(cell, positions, charges)
    C = prep["C"]
    XP = prep["XP"]
    if C not in _cache:
        _cache[C] = build_program(C)
    nc = _cache[C]

    in_maps = []
    for core in range(N_CORES):
        ch, h = divmod(core, 2)
        in_maps.append({
            "spx": prep["SPX"][(ch, h)],
            "gx": prep["GX"],
            "gy": prep["GY"][h],
            "fxc": prep["Fxc"], "fxs": prep["Fxs"], "fxns": prep["Fxns"],
            "fict": prep["Fict"], "fist": prep["Fist"], "finst": prep["Finst"],
            "gt": prep["GTS"][h],
            "idn": prep["idn"],
        })
    res = run_bass_kernel_spmd(nc, in_maps, list(range(N_CORES)), trace=trace)

    NA, NSP = prep["NA"], prep["NSP"]
    pot = np.zeros((NA, NSP), dtype=np.float64)
    valid = prep["valid"]
    atom_flat = prep["atom_of"][valid]
    for core in range(N_CORES):
        ch, h = divmod(core, 2)
        out = res.results[core]["out"]          # (128, NS//XP): row xi*C+s, col g
        out_cs = out.reshape(XP, C, NS // XP).transpose(2, 0, 1).reshape(NS, C)
        np.add.at(pot[:, ch], atom_flat, out_cs[valid])
    return pot.astype(np.float32), res


def kernel(cell, positions, charges):
    pot, _ = _run(cell, positions, charges, trace=False)
    return pot


# revision 29
# speedup vs baseline: 1.0294x; 1.0019x over previous
"""MeshPotential (P3M-style) Trainium2 kernel — banded-spectrum version.

Key physics: with atomic smearing 0.4 the k-space kernel G ~ exp(-0.0079 n^2)
is < 1e-7 outside integer frequencies |n| <= 32.  So only a 64 x 64 x 32
band of the 256^3 rfft spectrum matters (verified: truncation rel err 4e-6).

Per-core (8 cores SPMD, core = (channel, ky-half)) the pipeline is fully
analytic in y and z (per-atom structure factors, host-precomputed) and a
dense 256-point DFT in x only:

  P12  spread     : R(kzri, ky | x) = sum_slots a*wx*[SzR|SzI] (x) Sy
  T1   transpose  : [kzri, x] -> [x, kzri] blocks (PE transpose)
  P3   x-DFT + G  : X(kx, ky, kzri) = F_x R ;  X *= G   (banded kx: 64)
  P4   x-inverse  : V(x, ky, kzri)  = F_x^H X
  T2   transpose  : [x, kzri] -> [kzri, x] blocks
  P56  gather     : U(slot, ky) = M^T V ; pot(slot|x) = sum_ky SyPack * U

All matmul operands are bf16 (PSUM accumulates fp32).  Whole spectral cube
lives in SBUF (~5 MB); no DRAM round trips.  Host folds bin slots back to
atoms and sums the two ky-half cores per channel.
"""

import os

import numpy as np
import ml_dtypes

import concourse.bass as bass
import concourse.mybir as mybir
import concourse.tile as tile
from concourse import bacc
from concourse.bass_utils import run_bass_kernel_spmd

F32 = mybir.dt.float32
BF16 = mybir.dt.bfloat16
BFNP = ml_dtypes.bfloat16

NS = 256
BK = 64            # kx / ky band size (freqs 0..31, -32..-1)
KZB = 32           # kz band size (0..31)
KRIB = 2 * KZB     # [Re | Im] packed kz
KYH = 32           # ky values per core (half of band)
N_CORES = 8
SMEARING = 0.4
BOX_REF = None     # general cell handled via inv_cell in host_prep

_cache = {}


def build_program(C):
    XP = 128 // C                  # x cells per spread/gather sub-group
    G2 = NS // (2 * XP)            # gather batches (2 groups each)
    nc = bacc.Bacc(None, target_bir_lowering=False, debug=False)
    dp = lambda name, shape, dt=BF16: nc.declare_dram_parameter(
        name, list(shape), dt, isOutput=False)
    ST = max(1, 128 // (2 * C))    # x cells stacked per spread matmul
    XG = 16                        # x cells per spread DMA group
    NPAIR = XG // ST
    SW = 64 + 32 * ST
    spx = dp("spx", (NS // XG, 2 * C * ST, NPAIR, SW))   # [L-stack | blockdiag R]
    gx = dp("gx", (G2, BK, 2, 2, XP * C))          # [M1-all-xi | M2-all-xi]
    gy = dp("gy", (G2, 128, 2, 2, KYH, XP))        # diag-masked SyPack
    fxc = dp("fxc", (NS, BK))
    fxs = dp("fxs", (NS, BK))
    fxns = dp("fxns", (NS, BK))
    fict = dp("fict", (BK, NS))
    fist = dp("fist", (BK, NS))
    finst = dp("finst", (BK, NS))
    gt = dp("gt", (BK, KYH, KZB))                  # G/det for own ky half
    idn = dp("idn", (128, 128))
    outp = nc.declare_dram_parameter("out", [128, NS // XP], F32, isOutput=True)
    mult = mybir.AluOpType.mult
    add = mybir.AluOpType.add

    with tile.TileContext(nc) as tc:
        with (
            tc.tile_pool(name="constp", bufs=1) as constp,
            tc.tile_pool(name="iop", bufs=8) as iop,
            tc.tile_pool(name="psp", bufs=4, space="PSUM") as psp,
        ):
            # issue the first spread loads before the constants so P12 can
            # start the moment data lands
            spt_pre = []
            for g4 in range(2):
                spt = iop.tile([2 * C * ST, NPAIR, SW], BF16, tag="spt")
                nc.sync.dma_start(spt[:], spx[g4])
                spt_pre.append(spt)
            FXC = constp.tile([128, 2, BK], BF16)
            FXS = constp.tile([128, 2, BK], BF16)
            FXNS = constp.tile([128, 2, BK], BF16)
            for ch in range(2):
                nc.sync.dma_start(FXC[:, ch], fxc[128 * ch:128 * (ch + 1), :])
                nc.sync.dma_start(FXS[:, ch], fxs[128 * ch:128 * (ch + 1), :])
                nc.sync.dma_start(FXNS[:, ch], fxns[128 * ch:128 * (ch + 1), :])
            FICT = constp.tile([BK, NS], BF16)
            FIST = constp.tile([BK, NS], BF16)
            FINST = constp.tile([BK, NS], BF16)
            nc.sync.dma_start(FICT[:], fict[:])
            nc.sync.dma_start(FIST[:], fist[:])
            nc.sync.dma_start(FINST[:], finst[:])
            GT = constp.tile([BK, KYH, KZB], BF16)
            nc.sync.dma_start(GT[:], gt[:])
            IDN = constp.tile([128, 128], BF16)
            nc.sync.dma_start(IDN[:], idn[:])
            OUT = constp.tile([128, NS // XP], F32)

            # SBUF-resident spectral cubes (bf16)
            CB2 = constp.tile([BK, NS, KYH], BF16)          # (kzri, x, ky)
            CB3 = constp.tile([128, 2, KYH, KRIB], BF16)    # (x, xch, ky, kzri)
            CB4 = constp.tile([BK, KYH, KRIB], BF16)        # (kx, ky, kzri)
            CB5 = constp.tile([128, 2, KYH, KRIB], BF16)    # (x, xch, ky, kzri)
            CB6 = constp.tile([BK, KYH, NS], BF16)          # (kzri, ky, x)

            # ---------------- P12: spread (analytic y,z) ----------------
            # ST x cells share one matmul: lhsT stacks their [L1;L2] blocks on
            # the contract dim; rhs is block-diagonal so outputs stay separate
            for g4 in range(NS // XG):
                if g4 < 2:
                    spt = spt_pre[g4]
                else:
                    spt = iop.tile([2 * C * ST, NPAIR, SW], BF16, tag="spt")
                    nc.sync.dma_start(spt[:], spx[g4])
                ps = psp.tile([BK, NPAIR, ST, KYH], F32, tag="A")
                for p in range(NPAIR):
                    nc.tensor.matmul(ps[:, p], spt[:, p, 0:64],
                                     spt[:, p, 64:SW], start=True, stop=True)
                x0 = g4 * XG
                if g4 % 2 == 0:
                    nc.scalar.copy(CB2[:, x0:x0 + XG, :], ps[:])
                else:
                    nc.vector.tensor_copy(CB2[:, x0:x0 + XG, :], ps[:])

            # ---------------- T1: (kzri, x) -> (x, kzri) ----------------
            for ky0 in range(0, KYH, 8):
                for xch in range(2):
                    xsl = slice(128 * xch, 128 * (xch + 1))
                    pst = psp.tile([128, 8, BK], BF16, tag="B")
                    for i in range(8):
                        nc.tensor.transpose(pst[:, i, :], CB2[:, xsl, ky0 + i],
                                            IDN[0:BK, 0:BK])
                    if (ky0 // 8 + xch) % 2 == 0:
                        nc.scalar.copy(CB3[:, xch, ky0:ky0 + 8, :], pst[:])
                    else:
                        nc.vector.tensor_copy(CB3[:, xch, ky0:ky0 + 8, :], pst[:])

            # ---------------- P3: x-DFT (banded) + G ---------------------
            for kyg in range(0, KYH, 16):
                ksl = slice(kyg, kyg + 16)
                pxr = psp.tile([BK, 16, KZB], F32, tag="A")
                pxi = psp.tile([BK, 16, KZB], F32, tag="B")
                # XR = Fxc@CR + Fxs@CI ; XI = Fxc@CI - Fxs@CR
                nc.tensor.matmul(pxr[:], FXC[:, 0], CB3[:, 0, ksl, 0:KZB], start=True, stop=False)
                nc.tensor.matmul(pxr[:], FXC[:, 1], CB3[:, 1, ksl, 0:KZB], start=False, stop=False)
                nc.tensor.matmul(pxr[:], FXS[:, 0], CB3[:, 0, ksl, KZB:KRIB], start=False, stop=False)
                nc.tensor.matmul(pxr[:], FXS[:, 1], CB3[:, 1, ksl, KZB:KRIB], start=False, stop=True)
                nc.tensor.matmul(pxi[:], FXC[:, 0], CB3[:, 0, ksl, KZB:KRIB], start=True, stop=False)
                nc.tensor.matmul(pxi[:], FXC[:, 1], CB3[:, 1, ksl, KZB:KRIB], start=False, stop=False)
                nc.tensor.matmul(pxi[:], FXNS[:, 0], CB3[:, 0, ksl, 0:KZB], start=False, stop=False)
                nc.tensor.matmul(pxi[:], FXNS[:, 1], CB3[:, 1, ksl, 0:KZB], start=False, stop=True)
                nc.vector.tensor_tensor(CB4[:, ksl, 0:KZB], pxr[:], GT[:, ksl, :], op=mult)
                nc.vector.tensor_tensor(CB4[:, ksl, KZB:KRIB], pxi[:], GT[:, ksl, :], op=mult)

            # ---------------- P4: inverse x-DFT --------------------------
            for kyg in range(0, KYH, 16):
                ksl = slice(kyg, kyg + 16)
                for xch in range(2):
                    xsl = slice(128 * xch, 128 * (xch + 1))
                    pvr = psp.tile([128, 16, KZB], F32, tag="A")
                    pvi = psp.tile([128, 16, KZB], F32, tag="B")
                    # VR = Fic@XR - Fis@XI ; VI = Fis@XR + Fic@XI
                    nc.tensor.matmul(pvr[:], FICT[:, xsl], CB4[:, ksl, 0:KZB], start=True, stop=False)
                    nc.tensor.matmul(pvr[:], FINST[:, xsl], CB4[:, ksl, KZB:KRIB], start=False, stop=True)
                    nc.tensor.matmul(pvi[:], FIST[:, xsl], CB4[:, ksl, 0:KZB], start=True, stop=False)
                    nc.tensor.matmul(pvi[:], FICT[:, xsl], CB4[:, ksl, KZB:KRIB], start=False, stop=True)
                    nc.scalar.copy(CB5[:, xch, ksl, 0:KZB], pvr[:])
                    nc.vector.tensor_copy(CB5[:, xch, ksl, KZB:KRIB], pvi[:])

            # ---------------- T2: (x, kzri) -> (kzri, x) ----------------
            # xch outer: the x<128 half of CB6 completes first, so the gather
            # (which walks x in order) can start while xch=1 is still moving
            for xch in range(2):
                for ky0 in range(0, KYH, 8):
                    xsl = slice(128 * xch, 128 * (xch + 1))
                    pst = psp.tile([BK, 8, 128], BF16, tag="B")
                    for i in range(8):
                        nc.tensor.transpose(pst[:, i, :], CB5[:, xch, ky0 + i, :], IDN[:])
                    if (ky0 // 8 + xch) % 2 == 0:
                        nc.scalar.copy(CB6[:, ky0:ky0 + 8, xsl], pst[:])
                    else:
                        nc.vector.tensor_copy(CB6[:, ky0:ky0 + 8, xsl], pst[:])

            # ---------------- P56: gather (analytic y,z) -----------------
            # ps6[xi*C+s, gi, comp, ky, xj] = (M_comp of xi)^T V(xj); host-
            # baked gy mask keeps only xj == xi.  scalar evacuates PSUM,
            # gpsimd does the SyPack multiply, vector reduces.
            for q in range(G2 // 2):
                scr = iop.tile([128, 2, 2, 2, KYH, XP], F32, tag="scr")
                for half in range(2):
                    g2 = 2 * q + half
                    gmt = iop.tile([BK, 2, 2, XP * C], BF16, tag="gmt")
                    nc.sync.dma_start(gmt[:], gx[g2])
                    ps6 = psp.tile([128, 2, 2, KYH, XP], F32, tag="A")
                    for gi in range(2):
                        g = 2 * g2 + gi
                        for comp in range(2):
                            nc.tensor.matmul(ps6[:, gi, comp], gmt[:, gi, comp, :],
                                             CB6[:, :, g * XP:(g + 1) * XP],
                                             start=True, stop=True)
                    gyt = iop.tile([128, 2, 2, KYH, XP], BF16, tag="gyt")
                    nc.sync.dma_start(gyt[:], gy[g2])
                    if g2 % 2 == 0:
                        s6 = iop.tile([128, 2, 2, KYH, XP], F32, tag="s6")
                        nc.scalar.copy(s6[:], ps6[:])
                        nc.gpsimd.tensor_tensor(scr[:, half], s6[:], gyt[:], op=mult)
                    else:
                        nc.vector.tensor_tensor(scr[:, half], ps6[:], gyt[:], op=mult)
                nc.vector.tensor_reduce(OUT[:, 4 * q:4 * q + 4], scr[:],
                                        axis=mybir.AxisListType.XYZ, op=add)
                if q % 4 == 3:
                    nc.sync.dma_start(outp[:, 16 * (q // 4):16 * (q // 4) + 16],
                                      OUT[:, 16 * (q // 4):16 * (q // 4) + 16])
    nc.compile()
    return nc


def host_prep(cell, positions, charges):
    NA = positions.shape[0]
    NSP = charges.shape[1]
    cell = np.asarray(cell, dtype=np.float64)
    positions = np.asarray(positions, dtype=np.float64)
    charges = np.asarray(charges, dtype=np.float64)

    inv_cell = np.linalg.inv(cell)
    pos_rel = NS * (positions @ inv_cell)
    idx0 = np.floor(pos_rel)
    t = pos_rel - (idx0 + 0.5)
    t2 = t * t
    t3 = t2 * t
    w = np.stack([
        (1 - 6 * t + 12 * t2 - 8 * t3) / 48,
        (23 - 30 * t - 12 * t2 + 24 * t3) / 48,
        (23 + 30 * t - 12 * t2 - 24 * t3) / 48,
        (1 + 6 * t + 12 * t2 + 8 * t3) / 48,
    ])  # (4, NA, 3)
    offs = np.arange(-1, 3)
    idx = (idx0.astype(np.int64)[None] + offs[:, None, None]) % NS  # (4, NA, 3)

    nb = np.r_[0:BK // 2, -BK // 2:0].astype(np.float64)   # band freqs, fft order
    kzb = np.arange(KZB, dtype=np.float64)
    wkz = np.where(kzb == 0, 1.0, 2.0)

    # per-atom structure factors over the band
    Sy = np.zeros((NA, BK), dtype=np.complex128)
    Sz = np.zeros((NA, KZB), dtype=np.complex128)
    for j in range(4):
        Sy += w[j, :, 1:2] * np.exp(-2j * np.pi * np.outer(idx[j, :, 1], nb) / NS)
        Sz += w[j, :, 2:3] * np.exp(-2j * np.pi * np.outer(idx[j, :, 2], kzb) / NS)

    # bins over x cells
    entries = [[] for _ in range(NS)]
    for j in range(4):
        for n in range(NA):
            entries[idx[j, n, 0]].append((n, w[j, n, 0]))
    # snap bin capacity to {32, 64, 128} so per-x partition offsets (C*xi)
    # land on legal PE tile positions, and XP divides NS
    raw = max(len(e) for e in entries)
    assert raw <= 128, f"x-bin overflow: {raw}"
    C = 32 if raw <= 32 else (64 if raw <= 64 else 128)
    XP = 128 // C
    atom_of = np.zeros((NS, C), dtype=np.int64)
    wx_of = np.zeros((NS, C))
    valid = np.zeros((NS, C), dtype=bool)
    for x in range(NS):
        for s, (n, wx) in enumerate(entries[x]):
            atom_of[x, s] = n
            wx_of[x, s] = wx
            valid[x, s] = True

    SyB = Sy[atom_of]                    # (NS, C, BK)
    SzB = Sz[atom_of]                    # (NS, C, KZB)
    wv = (wx_of * valid)[..., None]

    # spread inputs per channel & ky-half, stacked on the contract dim:
    # rows [0:C] = (L1 | R1), rows [C:2C] = (L2 | R2), so one matmul per x
    SPX = {}
    for ch in range(NSP):
        a = (charges[atom_of, ch] * wx_of * valid)[..., None]
        L1 = np.concatenate([SzB.real, SzB.imag], -1) * a
        L2 = np.concatenate([-SzB.imag, SzB.real], -1) * a
        for h in range(2):
            ksl = slice(h * KYH, (h + 1) * KYH)
            Lst = np.concatenate([L1, L2], 1)                     # (NS,2C,64)
            Rst = np.concatenate(
                [SyB.real[:, :, ksl], SyB.imag[:, :, ksl]], 1)    # (NS,2C,32)
            ST = max(1, 128 // (2 * C))
            XG = 16
            SW = 64 + 32 * ST
            A = np.zeros((NS // ST, 2 * C * ST, SW))
            for s in range(ST):
                rows = slice(s * 2 * C, (s + 1) * 2 * C)
                A[:, rows, 0:64] = Lst[s::ST]
                A[:, rows, 64 + 32 * s:96 + 32 * s] = Rst[s::ST]
            SPX[(ch, h)] = np.ascontiguousarray(
                A.reshape(NS // XG, XG // ST, 2 * C * ST, SW)
                .transpose(0, 2, 1, 3)).astype(BFNP)

    # gather matrices (channel-independent): gx[g, kzri, comp, xi*C+s]
    M1 = (np.concatenate([SzB.real * wkz, SzB.imag * wkz], -1) * wv)   # (NS,C,64)
    M2 = (np.concatenate([-SzB.imag * wkz, SzB.real * wkz], -1) * wv)
    GX = np.ascontiguousarray(
        np.stack([M1, M2], 1)                      # (NS, 2, C, 64)
        .reshape(NS // (2 * XP), 2, XP, 2, C, BK).transpose(0, 5, 1, 3, 2, 4)
        .reshape(NS // (2 * XP), BK, 2, 2, XP * C)).astype(BFNP)

    # diag-masked SyPack: gy[g2, xi*C+s, gi, comp, xj, ky] = Sy_comp(s@x)*[xj==xi]
    GY = {}
    for h in range(2):
        ksl = slice(h * KYH, (h + 1) * KYH)
        syp = np.stack([SyB.real[:, :, ksl], SyB.imag[:, :, ksl]], 2) * valid[..., None, None]
        syg = syp.reshape(NS // (2 * XP), 2, XP, C, 2, KYH)   # (G2, gi, xi, s, comp, ky)
        g7 = np.zeros((NS // (2 * XP), 2, XP, C, 2, KYH, XP))
        for xi in range(XP):
            g7[:, :, xi, :, :, :, xi] = syg[:, :, xi]
        GY[h] = np.ascontiguousarray(
            g7.transpose(0, 2, 3, 1, 4, 5, 6)
            .reshape(NS // (2 * XP), XP * C, 2, 2, KYH, XP)).astype(BFNP)

    # banded Coulomb kernel / det
    recip = 2 * np.pi * inv_cell.T
    kxg, kyg, kzg = np.meshgrid(nb, nb, kzb, indexing="ij")
    kvec = kxg[..., None] * recip[0] + kyg[..., None] * recip[1] + kzg[..., None] * recip[2]
    ksq = np.sum(kvec * kvec, axis=-1)
    G = np.where(ksq == 0, 0.0,
                 4 * np.pi * np.exp(-0.5 * SMEARING**2 * ksq) / np.where(ksq == 0, 1.0, ksq))
    G = G / np.abs(np.linalg.det(cell))
    GTS = {h: np.ascontiguousarray(G[:, h * KYH:(h + 1) * KYH, :]).astype(BFNP)
           for h in range(2)}

    # band DFT matrices
    th = 2 * np.pi * np.outer(np.arange(NS), nb) / NS
    Fxc = np.cos(th).astype(BFNP)
    Fxs = np.sin(th).astype(BFNP)
    return dict(C=C, XP=XP, NSP=NSP, NA=NA, atom_of=atom_of, valid=valid,
                SPX=SPX, GX=GX, GY=GY, GTS=GTS,
                Fxc=Fxc, Fxs=Fxs, Fxns=(-Fxs).astype(BFNP),
                Fict=np.ascontiguousarray(Fxc.T), Fist=np.ascontiguousarray(Fxs.T),
                Finst=np.ascontiguousarray((-Fxs).astype(BFNP).T),
                idn=np.eye(128, dtype=BFNP))


def _run(cell, positions, charges, trace=False):
    prep = host_prep(cell, positions, charges)
    C = prep["C"]
    XP = prep["XP"]
    if C not in _cache:
        _cache[C] = build_program(C)
    nc = _cache[C]

    in_maps = []
    for core in range(N_CORES):
        ch, h = divmod(core, 2)
        in_maps.append({
            "spx": prep["SPX"][(ch, h)],
            "gx": prep["GX"],
            "gy": prep["GY"][h],
            "fxc": prep["Fxc"], "fxs": prep["Fxs"], "fxns": prep["Fxns"],
            "fict": prep["Fict"], "fist": prep["Fist"], "finst": prep["Finst"],
            "gt": prep["GTS"][h],
            "idn": prep["idn"],
        })
    res = run_bass_kernel_spmd(nc, in_maps, list(range(N_CORES)), trace=trace)

    NA, NSP = prep["NA"], prep["NSP"]
    pot = np.zeros((NA, NSP), dtype=np.float64)
    valid = prep["valid"]
    atom_flat = prep["atom_of"][valid]
    for core in range(N_CORES):
        ch, h = divmod(core, 2)
        out = res.results[core]["out"]          # (128, NS//XP): row xi*C+s, col g
        out_cs = out.reshape(XP, C, NS // XP).transpose(2, 0, 1).reshape(NS, C)
        np.add.at(pot[:, ch], atom_flat, out_cs[valid])
    return pot.astype(np.float32), res


def kernel(cell, positions, charges):
    pot, _ = _run(cell, positions, charges, trace=False)
    return pot


# revision 30
# speedup vs baseline: 1.0403x; 1.0106x over previous
"""MeshPotential (P3M-style) Trainium2 kernel — banded-spectrum version.

Key physics: with atomic smearing 0.4 the k-space kernel G ~ exp(-0.0079 n^2)
is < 1e-7 outside integer frequencies |n| <= 32.  So only a 64 x 64 x 32
band of the 256^3 rfft spectrum matters (verified: truncation rel err 4e-6).

Per-core (8 cores SPMD, core = (channel, ky-half)) the pipeline is fully
analytic in y and z (per-atom structure factors, host-precomputed) and a
dense 256-point DFT in x only:

  P12  spread     : R(kzri, ky | x) = sum_slots a*wx*[SzR|SzI] (x) Sy
  T1   transpose  : [kzri, x] -> [x, kzri] blocks (PE transpose)
  P3   x-DFT + G  : X(kx, ky, kzri) = F_x R ;  X *= G   (banded kx: 64)
  P4   x-inverse  : V(x, ky, kzri)  = F_x^H X
  T2   transpose  : [x, kzri] -> [kzri, x] blocks
  P56  gather     : U(slot, ky) = M^T V ; pot(slot|x) = sum_ky SyPack * U

All matmul operands are bf16 (PSUM accumulates fp32).  Whole spectral cube
lives in SBUF (~5 MB); no DRAM round trips.  Host folds bin slots back to
atoms and sums the two ky-half cores per channel.
"""

import os

import numpy as np
import ml_dtypes

import concourse.bass as bass
import concourse.mybir as mybir
import concourse.tile as tile
from concourse import bacc
from concourse.bass_utils import run_bass_kernel_spmd

F32 = mybir.dt.float32
BF16 = mybir.dt.bfloat16
BFNP = ml_dtypes.bfloat16

NS = 256
BK = 64            # kx / ky band size (freqs 0..31, -32..-1)
KZB = 32           # kz band size (0..31)
KRIB = 2 * KZB     # [Re | Im] packed kz
KYH = 32           # ky values per core (half of band)
N_CORES = 8
SMEARING = 0.4
BOX_REF = None     # general cell handled via inv_cell in host_prep

_cache = {}


def build_program(C):
    XP = 128 // C                  # x cells per spread/gather sub-group
    G2 = NS // (2 * XP)            # gather batches (2 groups each)
    nc = bacc.Bacc(None, target_bir_lowering=False, debug=False)
    dp = lambda name, shape, dt=BF16: nc.declare_dram_parameter(
        name, list(shape), dt, isOutput=False)
    ST = max(1, 128 // (2 * C))    # x cells stacked per spread matmul
    XG = 16                        # x cells per spread DMA group
    NPAIR = XG // ST
    SW = 64 + 32 * ST
    spx = dp("spx", (NS // XG, 2 * C * ST, NPAIR, SW))   # [L-stack | blockdiag R]
    gx = dp("gx", (G2, BK, 2, 2, XP * C))          # [M1-all-xi | M2-all-xi]
    gy = dp("gy", (G2, 128, 2, 2, KYH, XP))        # diag-masked SyPack
    fxc = dp("fxc", (NS, BK))
    fxs = dp("fxs", (NS, BK))
    fxns = dp("fxns", (NS, BK))
    fict = dp("fict", (BK, NS))
    fist = dp("fist", (BK, NS))
    finst = dp("finst", (BK, NS))
    gt = dp("gt", (BK, KYH, KZB))                  # G/det for own ky half
    idn = dp("idn", (128, 128))
    outp = nc.declare_dram_parameter("out", [128, NS // XP], F32, isOutput=True)
    mult = mybir.AluOpType.mult
    add = mybir.AluOpType.add

    with tile.TileContext(nc) as tc:
        with (
            tc.tile_pool(name="constp", bufs=1) as constp,
            tc.tile_pool(name="iop", bufs=8) as iop,
            tc.tile_pool(name="psp", bufs=4, space="PSUM") as psp,
        ):
            # issue the first spread loads before the constants so P12 can
            # start the moment data lands
            spt_pre = []
            for g4 in range(2):
                spt = iop.tile([2 * C * ST, NPAIR, SW], BF16, tag="spt")
                nc.sync.dma_start(spt[:], spx[g4])
                spt_pre.append(spt)
            FXC = constp.tile([128, 2, BK], BF16)
            FXS = constp.tile([128, 2, BK], BF16)
            FXNS = constp.tile([128, 2, BK], BF16)
            for ch in range(2):
                nc.sync.dma_start(FXC[:, ch], fxc[128 * ch:128 * (ch + 1), :])
                nc.sync.dma_start(FXS[:, ch], fxs[128 * ch:128 * (ch + 1), :])
                nc.sync.dma_start(FXNS[:, ch], fxns[128 * ch:128 * (ch + 1), :])
            FICT = constp.tile([BK, NS], BF16)
            FIST = constp.tile([BK, NS], BF16)
            FINST = constp.tile([BK, NS], BF16)
            nc.sync.dma_start(FICT[:], fict[:])
            nc.sync.dma_start(FIST[:], fist[:])
            nc.sync.dma_start(FINST[:], finst[:])
            GT = constp.tile([BK, KYH, KZB], BF16)
            nc.sync.dma_start(GT[:], gt[:])
            IDN = constp.tile([128, 128], BF16)
            nc.sync.dma_start(IDN[:], idn[:])
            OUT = constp.tile([128, NS // XP], F32)

            # SBUF-resident spectral cubes (bf16)
            CB2 = constp.tile([BK, NS, KYH], BF16)          # (kzri, x, ky)
            CB3 = constp.tile([128, 2, KYH, KRIB], BF16)    # (x, xch, ky, kzri)
            CB4 = constp.tile([BK, KYH, KRIB], BF16)        # (kx, ky, kzri)
            CB5 = constp.tile([128, 2, KYH, KRIB], BF16)    # (x, xch, ky, kzri)
            CB6 = constp.tile([BK, KYH, NS], BF16)          # (kzri, ky, x)

            # ---------------- P12: spread (analytic y,z) ----------------
            # ST x cells share one matmul: lhsT stacks their [L1;L2] blocks on
            # the contract dim; rhs is block-diagonal so outputs stay separate
            def p12_half(lo, hi):
                for g4 in range(lo, hi):
                    if g4 < 2:
                        spt = spt_pre[g4]
                    else:
                        spt = iop.tile([2 * C * ST, NPAIR, SW], BF16, tag="spt")
                        nc.sync.dma_start(spt[:], spx[g4])
                    ps = psp.tile([BK, NPAIR, ST, KYH], F32, tag="A")
                    for p in range(NPAIR):
                        nc.tensor.matmul(ps[:, p], spt[:, p, 0:64],
                                         spt[:, p, 64:SW], start=True, stop=True)
                    x0 = g4 * XG
                    if g4 % 2 == 0:
                        nc.scalar.copy(CB2[:, x0:x0 + XG, :], ps[:])
                    else:
                        nc.vector.tensor_copy(CB2[:, x0:x0 + XG, :], ps[:])

            # T1 interleaved: transpose each x-half as soon as its spread is
            # done, so PE transposes overlap the other half's spread DMAs
            def t1_half(xch):
                xsl = slice(128 * xch, 128 * (xch + 1))
                for ky0 in range(0, KYH, 8):
                    pst = psp.tile([128, 8, BK], BF16, tag="B")
                    for i in range(8):
                        nc.tensor.transpose(pst[:, i, :], CB2[:, xsl, ky0 + i],
                                            IDN[0:BK, 0:BK])
                    if (ky0 // 8 + xch) % 2 == 0:
                        nc.scalar.copy(CB3[:, xch, ky0:ky0 + 8, :], pst[:])
                    else:
                        nc.vector.tensor_copy(CB3[:, xch, ky0:ky0 + 8, :], pst[:])

            half = NS // XG // 2
            p12_half(0, half)
            t1_half(0)
            p12_half(half, NS // XG)
            t1_half(1)

            # ---------------- P3: x-DFT (banded) + G ---------------------
            for kyg in range(0, KYH, 16):
                ksl = slice(kyg, kyg + 16)
                pxr = psp.tile([BK, 16, KZB], F32, tag="A")
                pxi = psp.tile([BK, 16, KZB], F32, tag="B")
                # XR = Fxc@CR + Fxs@CI ; XI = Fxc@CI - Fxs@CR
                nc.tensor.matmul(pxr[:], FXC[:, 0], CB3[:, 0, ksl, 0:KZB], start=True, stop=False)
                nc.tensor.matmul(pxr[:], FXC[:, 1], CB3[:, 1, ksl, 0:KZB], start=False, stop=False)
                nc.tensor.matmul(pxr[:], FXS[:, 0], CB3[:, 0, ksl, KZB:KRIB], start=False, stop=False)
                nc.tensor.matmul(pxr[:], FXS[:, 1], CB3[:, 1, ksl, KZB:KRIB], start=False, stop=True)
                nc.tensor.matmul(pxi[:], FXC[:, 0], CB3[:, 0, ksl, KZB:KRIB], start=True, stop=False)
                nc.tensor.matmul(pxi[:], FXC[:, 1], CB3[:, 1, ksl, KZB:KRIB], start=False, stop=False)
                nc.tensor.matmul(pxi[:], FXNS[:, 0], CB3[:, 0, ksl, 0:KZB], start=False, stop=False)
                nc.tensor.matmul(pxi[:], FXNS[:, 1], CB3[:, 1, ksl, 0:KZB], start=False, stop=True)
                nc.vector.tensor_tensor(CB4[:, ksl, 0:KZB], pxr[:], GT[:, ksl, :], op=mult)
                nc.vector.tensor_tensor(CB4[:, ksl, KZB:KRIB], pxi[:], GT[:, ksl, :], op=mult)

            # ---------------- P4: inverse x-DFT --------------------------
            for kyg in range(0, KYH, 16):
                ksl = slice(kyg, kyg + 16)
                for xch in range(2):
                    xsl = slice(128 * xch, 128 * (xch + 1))
                    pvr = psp.tile([128, 16, KZB], F32, tag="A")
                    pvi = psp.tile([128, 16, KZB], F32, tag="B")
                    # VR = Fic@XR - Fis@XI ; VI = Fis@XR + Fic@XI
                    nc.tensor.matmul(pvr[:], FICT[:, xsl], CB4[:, ksl, 0:KZB], start=True, stop=False)
                    nc.tensor.matmul(pvr[:], FINST[:, xsl], CB4[:, ksl, KZB:KRIB], start=False, stop=True)
                    nc.tensor.matmul(pvi[:], FIST[:, xsl], CB4[:, ksl, 0:KZB], start=True, stop=False)
                    nc.tensor.matmul(pvi[:], FICT[:, xsl], CB4[:, ksl, KZB:KRIB], start=False, stop=True)
                    nc.scalar.copy(CB5[:, xch, ksl, 0:KZB], pvr[:])
                    nc.vector.tensor_copy(CB5[:, xch, ksl, KZB:KRIB], pvi[:])

            # ---------------- T2: (x, kzri) -> (kzri, x) ----------------
            # xch outer: the x<128 half of CB6 completes first, so the gather
            # (which walks x in order) can start while xch=1 is still moving
            for xch in range(2):
                for ky0 in range(0, KYH, 8):
                    xsl = slice(128 * xch, 128 * (xch + 1))
                    pst = psp.tile([BK, 8, 128], BF16, tag="B")
                    for i in range(8):
                        nc.tensor.transpose(pst[:, i, :], CB5[:, xch, ky0 + i, :], IDN[:])
                    if (ky0 // 8 + xch) % 2 == 0:
                        nc.scalar.copy(CB6[:, ky0:ky0 + 8, xsl], pst[:])
                    else:
                        nc.vector.tensor_copy(CB6[:, ky0:ky0 + 8, xsl], pst[:])

            # ---------------- P56: gather (analytic y,z) -----------------
            # ps6[xi*C+s, gi, comp, ky, xj] = (M_comp of xi)^T V(xj); host-
            # baked gy mask keeps only xj == xi.  scalar evacuates PSUM,
            # gpsimd does the SyPack multiply, vector reduces.
            for q in range(G2 // 2):
                scr = iop.tile([128, 2, 2, 2, KYH, XP], F32, tag="scr")
                for half in range(2):
                    g2 = 2 * q + half
                    gmt = iop.tile([BK, 2, 2, XP * C], BF16, tag="gmt")
                    nc.sync.dma_start(gmt[:], gx[g2])
                    ps6 = psp.tile([128, 2, 2, KYH, XP], F32, tag="A")
                    for gi in range(2):
                        g = 2 * g2 + gi
                        for comp in range(2):
                            nc.tensor.matmul(ps6[:, gi, comp], gmt[:, gi, comp, :],
                                             CB6[:, :, g * XP:(g + 1) * XP],
                                             start=True, stop=True)
                    gyt = iop.tile([128, 2, 2, KYH, XP], BF16, tag="gyt")
                    nc.sync.dma_start(gyt[:], gy[g2])
                    if g2 % 2 == 0:
                        s6 = iop.tile([128, 2, 2, KYH, XP], F32, tag="s6")
                        nc.scalar.copy(s6[:], ps6[:])
                        nc.gpsimd.tensor_tensor(scr[:, half], s6[:], gyt[:], op=mult)
                    else:
                        nc.vector.tensor_tensor(scr[:, half], ps6[:], gyt[:], op=mult)
                nc.vector.tensor_reduce(OUT[:, 4 * q:4 * q + 4], scr[:],
                                        axis=mybir.AxisListType.XYZ, op=add)
                if q % 4 == 3:
                    nc.sync.dma_start(outp[:, 16 * (q // 4):16 * (q // 4) + 16],
                                      OUT[:, 16 * (q // 4):16 * (q // 4) + 16])
    nc.compile()
    return nc


def host_prep(cell, positions, charges):
    NA = positions.shape[0]
    NSP = charges.shape[1]
    cell = np.asarray(cell, dtype=np.float64)
    positions = np.asarray(positions, dtype=np.float64)
    charges = np.asarray(charges, dtype=np.float64)

    inv_cell = np.linalg.inv(cell)
    pos_rel = NS * (positions @ inv_cell)
    idx0 = np.floor(pos_rel)
    t = pos_rel - (idx0 + 0.5)
    t2 = t * t
    t3 = t2 * t
    w = np.stack([
        (1 - 6 * t + 12 * t2 - 8 * t3) / 48,
        (23 - 30 * t - 12 * t2 + 24 * t3) / 48,
        (23 + 30 * t - 12 * t2 - 24 * t3) / 48,
        (1 + 6 * t + 12 * t2 + 8 * t3) / 48,
    ])  # (4, NA, 3)
    offs = np.arange(-1, 3)
    idx = (idx0.astype(np.int64)[None] + offs[:, None, None]) % NS  # (4, NA, 3)

    nb = np.r_[0:BK // 2, -BK // 2:0].astype(np.float64)   # band freqs, fft order
    kzb = np.arange(KZB, dtype=np.float64)
    wkz = np.where(kzb == 0, 1.0, 2.0)

    # per-atom structure factors over the band
    Sy = np.zeros((NA, BK), dtype=np.complex128)
    Sz = np.zeros((NA, KZB), dtype=np.complex128)
    for j in range(4):
        Sy += w[j, :, 1:2] * np.exp(-2j * np.pi * np.outer(idx[j, :, 1], nb) / NS)
        Sz += w[j, :, 2:3] * np.exp(-2j * np.pi * np.outer(idx[j, :, 2], kzb) / NS)

    # bins over x cells
    entries = [[] for _ in range(NS)]
    for j in range(4):
        for n in range(NA):
            entries[idx[j, n, 0]].append((n, w[j, n, 0]))
    # snap bin capacity to {32, 64, 128} so per-x partition offsets (C*xi)
    # land on legal PE tile positions, and XP divides NS
    raw = max(len(e) for e in entries)
    assert raw <= 128, f"x-bin overflow: {raw}"
    C = 32 if raw <= 32 else (64 if raw <= 64 else 128)
    XP = 128 // C
    atom_of = np.zeros((NS, C), dtype=np.int64)
    wx_of = np.zeros((NS, C))
    valid = np.zeros((NS, C), dtype=bool)
    for x in range(NS):
        for s, (n, wx) in enumerate(entries[x]):
            atom_of[x, s] = n
            wx_of[x, s] = wx
            valid[x, s] = True

    SyB = Sy[atom_of]                    # (NS, C, BK)
    SzB = Sz[atom_of]                    # (NS, C, KZB)
    wv = (wx_of * valid)[..., None]

    # spread inputs per channel & ky-half, stacked on the contract dim:
    # rows [0:C] = (L1 | R1), rows [C:2C] = (L2 | R2), so one matmul per x
    SPX = {}
    for ch in range(NSP):
        a = (charges[atom_of, ch] * wx_of * valid)[..., None]
        L1 = np.concatenate([SzB.real, SzB.imag], -1) * a
        L2 = np.concatenate([-SzB.imag, SzB.real], -1) * a
        for h in range(2):
            ksl = slice(h * KYH, (h + 1) * KYH)
            Lst = np.concatenate([L1, L2], 1)                     # (NS,2C,64)
            Rst = np.concatenate(
                [SyB.real[:, :, ksl], SyB.imag[:, :, ksl]], 1)    # (NS,2C,32)
            ST = max(1, 128 // (2 * C))
            XG = 16
            SW = 64 + 32 * ST
            A = np.zeros((NS // ST, 2 * C * ST, SW))
            for s in range(ST):
                rows = slice(s * 2 * C, (s + 1) * 2 * C)
                A[:, rows, 0:64] = Lst[s::ST]
                A[:, rows, 64 + 32 * s:96 + 32 * s] = Rst[s::ST]
            SPX[(ch, h)] = np.ascontiguousarray(
                A.reshape(NS // XG, XG // ST, 2 * C * ST, SW)
                .transpose(0, 2, 1, 3)).astype(BFNP)

    # gather matrices (channel-independent): gx[g, kzri, comp, xi*C+s]
    M1 = (np.concatenate([SzB.real * wkz, SzB.imag * wkz], -1) * wv)   # (NS,C,64)
    M2 = (np.concatenate([-SzB.imag * wkz, SzB.real * wkz], -1) * wv)
    GX = np.ascontiguousarray(
        np.stack([M1, M2], 1)                      # (NS, 2, C, 64)
        .reshape(NS // (2 * XP), 2, XP, 2, C, BK).transpose(0, 5, 1, 3, 2, 4)
        .reshape(NS // (2 * XP), BK, 2, 2, XP * C)).astype(BFNP)

    # diag-masked SyPack: gy[g2, xi*C+s, gi, comp, xj, ky] = Sy_comp(s@x)*[xj==xi]
    GY = {}
    for h in range(2):
        ksl = slice(h * KYH, (h + 1) * KYH)
        syp = np.stack([SyB.real[:, :, ksl], SyB.imag[:, :, ksl]], 2) * valid[..., None, None]
        syg = syp.reshape(NS // (2 * XP), 2, XP, C, 2, KYH)   # (G2, gi, xi, s, comp, ky)
        g7 = np.zeros((NS // (2 * XP), 2, XP, C, 2, KYH, XP))
        for xi in range(XP):
            g7[:, :, xi, :, :, :, xi] = syg[:, :, xi]
        GY[h] = np.ascontiguousarray(
            g7.transpose(0, 2, 3, 1, 4, 5, 6)
            .reshape(NS // (2 * XP), XP * C, 2, 2, KYH, XP)).astype(BFNP)

    # banded Coulomb kernel / det
    recip = 2 * np.pi * inv_cell.T
    kxg, kyg, kzg = np.meshgrid(nb, nb, kzb, indexing="ij")
    kvec = kxg[..., None] * recip[0] + kyg[..., None] * recip[1] + kzg[..., None] * recip[2]
    ksq = np.sum(kvec * kvec, axis=-1)
    G = np.where(ksq == 0, 0.0,
                 4 * np.pi * np.exp(-0.5 * SMEARING**2 * ksq) / np.where(ksq == 0, 1.0, ksq))
    G = G / np.abs(np.linalg.det(cell))
    GTS = {h: np.ascontiguousarray(G[:, h * KYH:(h + 1) * KYH, :]).astype(BFNP)
           for h in range(2)}

    # band DFT matrices
    th = 2 * np.pi * np.outer(np.arange(NS), nb) / NS
    Fxc = np.cos(th).astype(BFNP)
    Fxs = np.sin(th).astype(BFNP)
    return dict(C=C, XP=XP, NSP=NSP, NA=NA, atom_of=atom_of, valid=valid,
                SPX=SPX, GX=GX, GY=GY, GTS=GTS,
                Fxc=Fxc, Fxs=Fxs, Fxns=(-Fxs).astype(BFNP),
                Fict=np.ascontiguousarray(Fxc.T), Fist=np.ascontiguousarray(Fxs.T),
                Finst=np.ascontiguousarray((-Fxs).astype(BFNP).T),
                idn=np.eye(128, dtype=BFNP))


def _run(cell, positions, charges, trace=False):
    prep = host_prep(cell, positions, charges)
    C = prep["C"]
    XP = prep["XP"]
    if C not in _cache:
        _cache[C] = build_program(C)
    nc = _cache[C]

    in_maps = []
    for core in range(N_CORES):
        ch, h = divmod(core, 2)
        in_maps.append({
            "spx": prep["SPX"][(ch, h)],
            "gx": prep["GX"],
            "gy": prep["GY"][h],
            "fxc": prep["Fxc"], "fxs": prep["Fxs"], "fxns": prep["Fxns"],
            "fict": prep["Fict"], "fist": prep["Fist"], "finst": prep["Finst"],
            "gt": prep["GTS"][h],
            "idn": prep["idn"],
        })
    res = run_bass_kernel_spmd(nc, in_maps, list(range(N_CORES)), trace=trace)

    NA, NSP = prep["NA"], prep["NSP"]
    pot = np.zeros((NA, NSP), dtype=np.float64)
    valid = prep["valid"]
    atom_flat = prep["atom_of"][valid]
    for core in range(N_CORES):
        ch, h = divmod(core, 2)
        out = res.results[core]["out"]          # (128, NS//XP): row xi*C+s, col g
        out_cs = out.reshape(XP, C, NS // XP).transpose(2, 0, 1).reshape(NS, C)
        np.add.at(pot[:, ch], atom_flat, out_cs[valid])
    return pot.astype(np.float32), res


def kernel(cell, positions, charges):
    pot, _ = _run(cell, positions, charges, trace=False)
    return pot


# revision 31
# speedup vs baseline: 1.0776x; 1.0359x over previous
"""MeshPotential (P3M-style) Trainium2 kernel — banded-spectrum version.

Key physics: with atomic smearing 0.4 the k-space kernel G ~ exp(-0.0079 n^2)
is < 1e-7 outside integer frequencies |n| <= 32.  So only a 64 x 64 x 32
band of the 256^3 rfft spectrum matters (verified: truncation rel err 4e-6).

Per-core (8 cores SPMD, core = (channel, ky-half)) the pipeline is fully
analytic in y and z (per-atom structure factors, host-precomputed) and a
dense 256-point DFT in x only:

  P12  spread     : R(kzri, ky | x) = sum_slots a*wx*[SzR|SzI] (x) Sy
  T1   transpose  : [kzri, x] -> [x, kzri] blocks (PE transpose)
  P3   x-DFT + G  : X(kx, ky, kzri) = F_x R ;  X *= G   (banded kx: 64)
  P4   x-inverse  : V(x, ky, kzri)  = F_x^H X
  T2   transpose  : [x, kzri] -> [kzri, x] blocks
  P56  gather     : U(slot, ky) = M^T V ; pot(slot|x) = sum_ky SyPack * U

All matmul operands are bf16 (PSUM accumulates fp32).  Whole spectral cube
lives in SBUF (~5 MB); no DRAM round trips.  Host folds bin slots back to
atoms and sums the two ky-half cores per channel.
"""

import os

import numpy as np
import ml_dtypes

import concourse.bass as bass
import concourse.mybir as mybir
import concourse.tile as tile
from concourse import bacc
from concourse.bass_utils import run_bass_kernel_spmd

F32 = mybir.dt.float32
BF16 = mybir.dt.bfloat16
BFNP = ml_dtypes.bfloat16

NS = 256
BK = 64            # kx / ky band size (freqs 0..31, -32..-1)
KZB = 32           # kz band size (0..31)
KRIB = 2 * KZB     # [Re | Im] packed kz
KYH = 32           # ky values per core (half of band)
N_CORES = 8
SMEARING = 0.4
BOX_REF = None     # general cell handled via inv_cell in host_prep

_cache = {}


def build_program(C):
    XP = 128 // C                  # x cells per spread/gather sub-group
    G2 = NS // (2 * XP)            # gather batches (2 groups each)
    nc = bacc.Bacc(None, target_bir_lowering=False, debug=False)
    dp = lambda name, shape, dt=BF16: nc.declare_dram_parameter(
        name, list(shape), dt, isOutput=False)
    ST = max(1, 128 // (2 * C))    # x cells stacked per spread matmul
    XG = 16                        # x cells per spread DMA group
    NPAIR = XG // ST
    SW = 64 + 32 * ST
    spx = dp("spx", (NS // XG, 2 * C * ST, NPAIR, SW))   # [L-stack | blockdiag R]
    gx = dp("gx", (G2, BK, 2, 2, XP * C))          # [M1-all-xi | M2-all-xi]
    gy = dp("gy", (G2, 128, 2, 2, KYH, XP))        # diag-masked SyPack
    fxc = dp("fxc", (NS, BK))
    fxs = dp("fxs", (NS, BK))
    fxns = dp("fxns", (NS, BK))
    fict = dp("fict", (BK, NS))
    fist = dp("fist", (BK, NS))
    finst = dp("finst", (BK, NS))
    gt = dp("gt", (BK, KYH, KZB))                  # G/det for own ky half
    idn = dp("idn", (128, 128))
    outp = nc.declare_dram_parameter("out", [128, NS // XP], F32, isOutput=True)
    mult = mybir.AluOpType.mult
    add = mybir.AluOpType.add

    with tile.TileContext(nc) as tc:
        with (
            tc.tile_pool(name="constp", bufs=1) as constp,
            tc.tile_pool(name="iop", bufs=8) as iop,
            tc.tile_pool(name="psp", bufs=4, space="PSUM") as psp,
        ):
            # issue the first spread loads before the constants so P12 can
            # start the moment data lands
            spt_pre = []
            for g4 in range(2):
                spt = iop.tile([2 * C * ST, NPAIR, SW], BF16, tag="spt")
                nc.sync.dma_start(spt[:], spx[g4])
                spt_pre.append(spt)
            FXC = constp.tile([128, 2, BK], BF16)
            FXS = constp.tile([128, 2, BK], BF16)
            FXNS = constp.tile([128, 2, BK], BF16)
            for ch in range(2):
                nc.sync.dma_start(FXC[:, ch], fxc[128 * ch:128 * (ch + 1), :])
                nc.sync.dma_start(FXS[:, ch], fxs[128 * ch:128 * (ch + 1), :])
                nc.sync.dma_start(FXNS[:, ch], fxns[128 * ch:128 * (ch + 1), :])
            FICT = constp.tile([BK, NS], BF16)
            FIST = constp.tile([BK, NS], BF16)
            FINST = constp.tile([BK, NS], BF16)
            nc.sync.dma_start(FICT[:], fict[:])
            nc.sync.dma_start(FIST[:], fist[:])
            nc.sync.dma_start(FINST[:], finst[:])
            GT = constp.tile([BK, KYH, KZB], BF16)
            nc.sync.dma_start(GT[:], gt[:])
            IDN = constp.tile([128, 128], BF16)
            nc.sync.dma_start(IDN[:], idn[:])
            OUT = constp.tile([128, NS // XP], F32)

            # SBUF-resident spectral cubes (bf16)
            CB2 = constp.tile([BK, NS, KYH], BF16)          # (kzri, x, ky)
            CB3 = constp.tile([128, 2, KYH, KRIB], BF16)    # (x, xch, ky, kzri)
            CB4 = constp.tile([BK, KYH, KRIB], BF16)        # (kx, ky, kzri)
            CB5 = constp.tile([128, 2, KYH, KRIB], BF16)    # (x, xch, ky, kzri)
            CB6 = constp.tile([BK, KYH, NS], BF16)          # (kzri, ky, x)

            # ---------------- P12: spread (analytic y,z) ----------------
            # ST x cells share one matmul: lhsT stacks their [L1;L2] blocks on
            # the contract dim; rhs is block-diagonal so outputs stay separate
            def p12_half(lo, hi):
                for g4 in range(lo, hi):
                    if g4 < 2:
                        spt = spt_pre[g4]
                    else:
                        spt = iop.tile([2 * C * ST, NPAIR, SW], BF16, tag="spt")
                        nc.sync.dma_start(spt[:], spx[g4])
                    ps = psp.tile([BK, NPAIR, ST, KYH], F32, tag="A")
                    for p in range(NPAIR):
                        nc.tensor.matmul(ps[:, p], spt[:, p, 0:64],
                                         spt[:, p, 64:SW], start=True, stop=True)
                    x0 = g4 * XG
                    if g4 % 2 == 0:
                        nc.scalar.copy(CB2[:, x0:x0 + XG, :], ps[:])
                    else:
                        nc.vector.tensor_copy(CB2[:, x0:x0 + XG, :], ps[:])

            # T1 interleaved: transpose each x-half as soon as its spread is
            # done, so PE transposes overlap the other half's spread DMAs
            def t1_half(xch):
                xsl = slice(128 * xch, 128 * (xch + 1))
                for ky0 in range(0, KYH, 8):
                    pst = psp.tile([128, 8, BK], BF16, tag="B")
                    for i in range(8):
                        nc.tensor.transpose(pst[:, i, :], CB2[:, xsl, ky0 + i],
                                            IDN[0:BK, 0:BK])
                    if (ky0 // 8 + xch) % 2 == 0:
                        nc.scalar.copy(CB3[:, xch, ky0:ky0 + 8, :], pst[:])
                    else:
                        nc.vector.tensor_copy(CB3[:, xch, ky0:ky0 + 8, :], pst[:])

            half = NS // XG // 2
            p12_half(0, half)
            t1_half(0)
            p12_half(half, NS // XG)
            t1_half(1)

            # ---------------- P3: x-DFT (banded) + G ---------------------
            for kyg in range(0, KYH, 16):
                ksl = slice(kyg, kyg + 16)
                pxr = psp.tile([BK, 16, KZB], F32, tag="A")
                pxi = psp.tile([BK, 16, KZB], F32, tag="B")
                # XR = Fxc@CR + Fxs@CI ; XI = Fxc@CI - Fxs@CR
                nc.tensor.matmul(pxr[:], FXC[:, 0], CB3[:, 0, ksl, 0:KZB], start=True, stop=False)
                nc.tensor.matmul(pxr[:], FXC[:, 1], CB3[:, 1, ksl, 0:KZB], start=False, stop=False)
                nc.tensor.matmul(pxr[:], FXS[:, 0], CB3[:, 0, ksl, KZB:KRIB], start=False, stop=False)
                nc.tensor.matmul(pxr[:], FXS[:, 1], CB3[:, 1, ksl, KZB:KRIB], start=False, stop=True)
                nc.tensor.matmul(pxi[:], FXC[:, 0], CB3[:, 0, ksl, KZB:KRIB], start=True, stop=False)
                nc.tensor.matmul(pxi[:], FXC[:, 1], CB3[:, 1, ksl, KZB:KRIB], start=False, stop=False)
                nc.tensor.matmul(pxi[:], FXNS[:, 0], CB3[:, 0, ksl, 0:KZB], start=False, stop=False)
                nc.tensor.matmul(pxi[:], FXNS[:, 1], CB3[:, 1, ksl, 0:KZB], start=False, stop=True)
                nc.vector.tensor_tensor(CB4[:, ksl, 0:KZB], pxr[:], GT[:, ksl, :], op=mult)
                nc.vector.tensor_tensor(CB4[:, ksl, KZB:KRIB], pxi[:], GT[:, ksl, :], op=mult)

            # ---------------- P4: inverse x-DFT (per x-half) -------------
            def p4_half(xch):
                xsl = slice(128 * xch, 128 * (xch + 1))
                for kyg in range(0, KYH, 16):
                    ksl = slice(kyg, kyg + 16)
                    pvr = psp.tile([128, 16, KZB], F32, tag="A")
                    pvi = psp.tile([128, 16, KZB], F32, tag="B")
                    # VR = Fic@XR - Fis@XI ; VI = Fis@XR + Fic@XI
                    nc.tensor.matmul(pvr[:], FICT[:, xsl], CB4[:, ksl, 0:KZB], start=True, stop=False)
                    nc.tensor.matmul(pvr[:], FINST[:, xsl], CB4[:, ksl, KZB:KRIB], start=False, stop=True)
                    nc.tensor.matmul(pvi[:], FIST[:, xsl], CB4[:, ksl, 0:KZB], start=True, stop=False)
                    nc.tensor.matmul(pvi[:], FICT[:, xsl], CB4[:, ksl, KZB:KRIB], start=False, stop=True)
                    nc.scalar.copy(CB5[:, xch, ksl, 0:KZB], pvr[:])
                    nc.vector.tensor_copy(CB5[:, xch, ksl, KZB:KRIB], pvi[:])

            # ---------------- T2: (x, kzri) -> (kzri, x) per x-half ------
            def t2_half(xch):
                xsl = slice(128 * xch, 128 * (xch + 1))
                for ky0 in range(0, KYH, 8):
                    pst = psp.tile([BK, 8, 128], BF16, tag="B")
                    for i in range(8):
                        nc.tensor.transpose(pst[:, i, :], CB5[:, xch, ky0 + i, :], IDN[:])
                    if (ky0 // 8 + xch) % 2 == 0:
                        nc.scalar.copy(CB6[:, ky0:ky0 + 8, xsl], pst[:])
                    else:
                        nc.vector.tensor_copy(CB6[:, ky0:ky0 + 8, xsl], pst[:])

            # ---------------- P56: gather (analytic y,z) -----------------
            # ps6[xi*C+s, gi, comp, ky, xj] = (M_comp of xi)^T V(xj); host-
            # baked gy mask keeps only xj == xi.  scalar evacuates PSUM,
            # gpsimd does the SyPack multiply, vector reduces.
            def p56_range(q0, q1):
              for q in range(q0, q1):
                scr = iop.tile([128, 2, 2, 2, KYH, XP], F32, tag="scr")
                for half in range(2):
                    g2 = 2 * q + half
                    gmt = iop.tile([BK, 2, 2, XP * C], BF16, tag="gmt")
                    nc.sync.dma_start(gmt[:], gx[g2])
                    ps6 = psp.tile([128, 2, 2, KYH, XP], F32, tag="A")
                    for gi in range(2):
                        g = 2 * g2 + gi
                        for comp in range(2):
                            nc.tensor.matmul(ps6[:, gi, comp], gmt[:, gi, comp, :],
                                             CB6[:, :, g * XP:(g + 1) * XP],
                                             start=True, stop=True)
                    gyt = iop.tile([128, 2, 2, KYH, XP], BF16, tag="gyt")
                    nc.sync.dma_start(gyt[:], gy[g2])
                    if g2 % 2 == 0:
                        s6 = iop.tile([128, 2, 2, KYH, XP], F32, tag="s6")
                        nc.scalar.copy(s6[:], ps6[:])
                        nc.gpsimd.tensor_tensor(scr[:, half], s6[:], gyt[:], op=mult)
                    else:
                        nc.vector.tensor_tensor(scr[:, half], ps6[:], gyt[:], op=mult)
                nc.vector.tensor_reduce(OUT[:, 4 * q:4 * q + 4], scr[:],
                                        axis=mybir.AxisListType.XYZ, op=add)
                if q % 4 == 3:
                    nc.sync.dma_start(outp[:, 16 * (q // 4):16 * (q // 4) + 16],
                                      OUT[:, 16 * (q // 4):16 * (q // 4) + 16])

            p4_half(0)
            t2_half(0)
            p56_range(0, G2 // 4)
            p4_half(1)
            t2_half(1)
            p56_range(G2 // 4, G2 // 2)
    nc.compile()
    return nc


def host_prep(cell, positions, charges):
    NA = positions.shape[0]
    NSP = charges.shape[1]
    cell = np.asarray(cell, dtype=np.float64)
    positions = np.asarray(positions, dtype=np.float64)
    charges = np.asarray(charges, dtype=np.float64)

    inv_cell = np.linalg.inv(cell)
    pos_rel = NS * (positions @ inv_cell)
    idx0 = np.floor(pos_rel)
    t = pos_rel - (idx0 + 0.5)
    t2 = t * t
    t3 = t2 * t
    w = np.stack([
        (1 - 6 * t + 12 * t2 - 8 * t3) / 48,
        (23 - 30 * t - 12 * t2 + 24 * t3) / 48,
        (23 + 30 * t - 12 * t2 - 24 * t3) / 48,
        (1 + 6 * t + 12 * t2 + 8 * t3) / 48,
    ])  # (4, NA, 3)
    offs = np.arange(-1, 3)
    idx = (idx0.astype(np.int64)[None] + offs[:, None, None]) % NS  # (4, NA, 3)

    nb = np.r_[0:BK // 2, -BK // 2:0].astype(np.float64)   # band freqs, fft order
    kzb = np.arange(KZB, dtype=np.float64)
    wkz = np.where(kzb == 0, 1.0, 2.0)

    # per-atom structure factors over the band
    Sy = np.zeros((NA, BK), dtype=np.complex128)
    Sz = np.zeros((NA, KZB), dtype=np.complex128)
    for j in range(4):
        Sy += w[j, :, 1:2] * np.exp(-2j * np.pi * np.outer(idx[j, :, 1], nb) / NS)
        Sz += w[j, :, 2:3] * np.exp(-2j * np.pi * np.outer(idx[j, :, 2], kzb) / NS)

    # bins over x cells
    entries = [[] for _ in range(NS)]
    for j in range(4):
        for n in range(NA):
            entries[idx[j, n, 0]].append((n, w[j, n, 0]))
    # snap bin capacity to {32, 64, 128} so per-x partition offsets (C*xi)
    # land on legal PE tile positions, and XP divides NS
    raw = max(len(e) for e in entries)
    assert raw <= 128, f"x-bin overflow: {raw}"
    C = 32 if raw <= 32 else (64 if raw <= 64 else 128)
    XP = 128 // C
    atom_of = np.zeros((NS, C), dtype=np.int64)
    wx_of = np.zeros((NS, C))
    valid = np.zeros((NS, C), dtype=bool)
    for x in range(NS):
        for s, (n, wx) in enumerate(entries[x]):
            atom_of[x, s] = n
            wx_of[x, s] = wx
            valid[x, s] = True

    SyB = Sy[atom_of]                    # (NS, C, BK)
    SzB = Sz[atom_of]                    # (NS, C, KZB)
    wv = (wx_of * valid)[..., None]

    # spread inputs per channel & ky-half, stacked on the contract dim:
    # rows [0:C] = (L1 | R1), rows [C:2C] = (L2 | R2), so one matmul per x
    SPX = {}
    for ch in range(NSP):
        a = (charges[atom_of, ch] * wx_of * valid)[..., None]
        L1 = np.concatenate([SzB.real, SzB.imag], -1) * a
        L2 = np.concatenate([-SzB.imag, SzB.real], -1) * a
        for h in range(2):
            ksl = slice(h * KYH, (h + 1) * KYH)
            Lst = np.concatenate([L1, L2], 1)                     # (NS,2C,64)
            Rst = np.concatenate(
                [SyB.real[:, :, ksl], SyB.imag[:, :, ksl]], 1)    # (NS,2C,32)
            ST = max(1, 128 // (2 * C))
            XG = 16
            SW = 64 + 32 * ST
            A = np.zeros((NS // ST, 2 * C * ST, SW))
            for s in range(ST):
                rows = slice(s * 2 * C, (s + 1) * 2 * C)
                A[:, rows, 0:64] = Lst[s::ST]
                A[:, rows, 64 + 32 * s:96 + 32 * s] = Rst[s::ST]
            SPX[(ch, h)] = np.ascontiguousarray(
                A.reshape(NS // XG, XG // ST, 2 * C * ST, SW)
                .transpose(0, 2, 1, 3)).astype(BFNP)

    # gather matrices (channel-independent): gx[g, kzri, comp, xi*C+s]
    M1 = (np.concatenate([SzB.real * wkz, SzB.imag * wkz], -1) * wv)   # (NS,C,64)
    M2 = (np.concatenate([-SzB.imag * wkz, SzB.real * wkz], -1) * wv)
    GX = np.ascontiguousarray(
        np.stack([M1, M2], 1)                      # (NS, 2, C, 64)
        .reshape(NS // (2 * XP), 2, XP, 2, C, BK).transpose(0, 5, 1, 3, 2, 4)
        .reshape(NS // (2 * XP), BK, 2, 2, XP * C)).astype(BFNP)

    # diag-masked SyPack: gy[g2, xi*C+s, gi, comp, xj, ky] = Sy_comp(s@x)*[xj==xi]
    GY = {}
    for h in range(2):
        ksl = slice(h * KYH, (h + 1) * KYH)
        syp = np.stack([SyB.real[:, :, ksl], SyB.imag[:, :, ksl]], 2) * valid[..., None, None]
        syg = syp.reshape(NS // (2 * XP), 2, XP, C, 2, KYH)   # (G2, gi, xi, s, comp, ky)
        g7 = np.zeros((NS // (2 * XP), 2, XP, C, 2, KYH, XP))
        for xi in range(XP):
            g7[:, :, xi, :, :, :, xi] = syg[:, :, xi]
        GY[h] = np.ascontiguousarray(
            g7.transpose(0, 2, 3, 1, 4, 5, 6)
            .reshape(NS // (2 * XP), XP * C, 2, 2, KYH, XP)).astype(BFNP)

    # banded Coulomb kernel / det
    recip = 2 * np.pi * inv_cell.T
    kxg, kyg, kzg = np.meshgrid(nb, nb, kzb, indexing="ij")
    kvec = kxg[..., None] * recip[0] + kyg[..., None] * recip[1] + kzg[..., None] * recip[2]
    ksq = np.sum(kvec * kvec, axis=-1)
    G = np.where(ksq == 0, 0.0,
                 4 * np.pi * np.exp(-0.5 * SMEARING**2 * ksq) / np.where(ksq == 0, 1.0, ksq))
    G = G / np.abs(np.linalg.det(cell))
    GTS = {h: np.ascontiguousarray(G[:, h * KYH:(h + 1) * KYH, :]).astype(BFNP)
           for h in range(2)}

    # band DFT matrices
    th = 2 * np.pi * np.outer(np.arange(NS), nb) / NS
    Fxc = np.cos(th).astype(BFNP)
    Fxs = np.sin(th).astype(BFNP)
    return dict(C=C, XP=XP, NSP=NSP, NA=NA, atom_of=atom_of, valid=valid,
                SPX=SPX, GX=GX, GY=GY, GTS=GTS,
                Fxc=Fxc, Fxs=Fxs, Fxns=(-Fxs).astype(BFNP),
                Fict=np.ascontiguousarray(Fxc.T), Fist=np.ascontiguousarray(Fxs.T),
                Finst=np.ascontiguousarray((-Fxs).astype(BFNP).T),
                idn=np.eye(128, dtype=BFNP))


def _run(cell, positions, charges, trace=False):
    prep = host_prep(cell, positions, charges)
    C = prep["C"]
    XP = prep["XP"]
    if C not in _cache:
        _cache[C] = build_program(C)
    nc = _cache[C]

    in_maps = []
    for core in range(N_CORES):
        ch, h = divmod(core, 2)
        in_maps.append({
            "spx": prep["SPX"][(ch, h)],
            "gx": prep["GX"],
            "gy": prep["GY"][h],
            "fxc": prep["Fxc"], "fxs": prep["Fxs"], "fxns": prep["Fxns"],
            "fict": prep["Fict"], "fist": prep["Fist"], "finst": prep["Finst"],
            "gt": prep["GTS"][h],
            "idn": prep["idn"],
        })
    res = run_bass_kernel_spmd(nc, in_maps, list(range(N_CORES)), trace=trace)

    NA, NSP = prep["NA"], prep["NSP"]
    pot = np.zeros((NA, NSP), dtype=np.float64)
    valid = prep["valid"]
    atom_flat = prep["atom_of"][valid]
    for core in range(N_CORES):
        ch, h = divmod(core, 2)
        out = res.results[core]["out"]          # (128, NS//XP): row xi*C+s, col g
        out_cs = out.reshape(XP, C, NS // XP).transpose(2, 0, 1).reshape(NS, C)
        np.add.at(pot[:, ch], atom_flat, out_cs[valid])
    return pot.astype(np.float32), res


def kernel(cell, positions, charges):
    pot, _ = _run(cell, positions, charges, trace=False)
    return pot
